# revision 5
# baseline (speedup 1.0000x reference)
"""Bamba attention decoder layer on 8 Trainium2 NeuronCores.

Sharding: tensor-parallel attention (4 q heads + 1 kv head per core),
AllToAll of attention context (delivers each core its token slice at a static
address), token-sliced o_proj + fused add/rmsnorm, AllGather of normed
activations, I-sharded SwiGLU MLP (1792 cols/core), ReduceScatter of
down-proj partials.

Layout: feature-major activations ([features->partitions, tokens->free]) so
every linear layer uses its natural-layout weight block as the stationary
matmul operand. All activations/weights are fp16 (psum accumulation stays
fp32): halves DMA traffic and doubles DVE element throughput at unchanged PE
rate. The 1/rms factor of rmsnorm1 is applied after the QKV matmul (per-token
column scaling commutes through the contraction); ln weights are folded into
the weight matrices on the host.

Schedule notes (vs the original fp32 version):
- qkv weight chunks are interleaved with the first token block's hb loads so
  the PE starts ~5us in instead of waiting for the full 12.6MB.
- all 6 qkv psum banks are evacuated by immediate scalar-engine copies; the
  rmsnorm/rope chain runs from SBUF off the PE critical path, and the V
  transposes are deferred to a mini-phase after the last block.
- attention processes score tiles in pairs ([128,1024] psum tiles) to halve
  exp/mask/accumulate instruction count; probs in fp16.
- o_proj weights (fp16) stream under the matmul; the first 4 m-tiles and the
  residual slice prefetch during attention.
- the MLP first matmul is single-pass (all 2048 tokens resident in fp16), so
  gate/up weights load once; h round-trips DRAM in fp16 with the phase-5
  reload chunked per k-tile so it overlaps phase 4.
"""

import numpy as np

import concourse.bacc as bacc
import concourse.mybir as mybir
import concourse.tile as tile
from concourse.bass_utils import run_bass_kernel_spmd
from concourse.masks import make_identity

NC = 8
S = 2048
H = 4096
HD = 128
NQ = 32
NKV = 8
I = 14336
QH = NQ // NC        # q heads per core = 4
IPC = I // NC        # intermediate cols per core = 1792
TPC = S // NC        # tokens per core = 256
EPS = 1e-5
THETA = 10000.0
SCALE = HD ** -0.5

F32 = mybir.dt.float32
F16 = mybir.dt.float16

KH = H // 128        # 32 k-tiles over H
NB = S // 512        # 4 token blocks of 512
MB_GU = IPC // 128   # 14 m tiles for gate (and for up)
KI = IPC // 128      # 14 k tiles over I per core

AF = mybir.ActivationFunctionType


def _phase1_qkv(nc, tc, g):
    """QKV matmul + rmsnorm1 stats + rope. Fills qT_sb/kT_sb/vT_sb."""
    with (
        tc.tile_pool(name="p1sbuf", bufs=2) as p1s,
        tc.tile_pool(name="p1w", bufs=1) as p1w,
        tc.tile_pool(name="p1psum", bufs=1, space="PSUM") as p1p,
    ):
        wq_sb = p1w.tile([128, KH, (QH + 2) * 128], F16, name="wq_sb")  # 6.3 MB

        for nb in range(NB):
            ncols = slice(nb * 512, (nb + 1) * 512)
            st_ps = p1p.tile([1, 512], F32, name="st_ps", tag="st_ps")
            mm_ps = p1p.tile([128, (QH + 2), 512], F32, name="mm_ps", tag="mm_ps")
            for k in range(KH):
                if nb == 0:
                    # interleave weight-chunk loads with the first block's
                    # hb loads so the first matmuls start almost immediately
                    nc.sync.dma_start(wq_sb[:, k, :], g["wqkv"][:, k, :])
                hb = p1s.tile([128, 512], F16, name="hb", tag="hb", bufs=3)
                nc.sync.dma_start(hb[:], g["hT"][k * 128:(k + 1) * 128, ncols])
                sq = p1s.tile([128, 512], F16, name="sq", tag="sq", bufs=2)
                nc.scalar.activation(sq[:], hb[:], AF.Square)
                nc.tensor.matmul(st_ps[:], g["ones"][:], sq[:],
                                 start=(k == 0), stop=(k == KH - 1))
                for m in range(QH + 2):
                    nc.tensor.matmul(
                        mm_ps[:, m, :], wq_sb[:, k, m * 128:(m + 1) * 128], hb[:],
                        start=(k == 0), stop=(k == KH - 1),
                    )
                if nb == 0 and k == 20:
                    # small constants, needed at the end of block 0
                    nc.sync.dma_start(g["cos_sb"][:], g["cosT"][:, :])
                    nc.sync.dma_start(g["sin_sb"][:], g["sinT"][:, :])
                    nc.sync.dma_start(g["mask_sb"][:], g["masks"][:, :, :])
            # evacuate all 6 psum banks immediately (scalar engine) so the
            # next block's matmuls only wait on these copies, not on the
            # rmsnorm/rope chain below
            qkc = p1s.tile([128, QH + 2, 512], F16, name="qkc", tag="qkc", bufs=1)
            for m in range(QH + 2):
                nc.scalar.copy(qkc[:, m, :], mm_ps[:, m, :])
            std_row = p1s.tile([1, 512], F32, name="std_row", tag="std_row")
            nc.scalar.activation(std_row[:], st_ps[:], AF.Sqrt,
                                 bias=g["epsb"][:], scale=1.0 / H)
            rstd_row = p1s.tile([1, 512], F32, name="rstd_row", tag="rstd_row")
            nc.vector.reciprocal(rstd_row[:], std_row[:])
            rstd16 = p1s.tile([1, 512], F16, name="rstd16", tag="rstd16")
            nc.vector.tensor_copy(rstd16[:], rstd_row[:])
            rb = p1s.tile([128, 512], F16, name="rb", tag="rb", bufs=1)
            nc.gpsimd.partition_broadcast(rb[:], rstd16[:])
            cos_s = p1s.tile([128, 512], F16, name="cos_s", tag="cos_s", bufs=1)
            nc.vector.tensor_mul(cos_s[:], g["cos_sb"][:, ncols], rb[:])
            sin_s = p1s.tile([128, 512], F16, name="sin_s", tag="sin_s", bufs=1)
            nc.vector.tensor_mul(sin_s[:], g["sin_sb"][:, ncols], rb[:])
            for m in range(QH + 1):
                if m < QH:
                    d0 = g["qT_sb"][0:64, m, ncols]
                    d1 = g["qT_sb"][64:128, m, ncols]
                else:
                    d0 = g["kT_sb"][0:64, ncols]
                    d1 = g["kT_sb"][64:128, ncols]
                t0 = p1s.tile([64, 512], F16, name="t0", tag="t0", bufs=1)
                nc.vector.tensor_mul(t0[:], qkc[0:64, m, :], cos_s[0:64, :])
                t1 = p1s.tile([64, 512], F16, name="t1", tag="t1", bufs=1)
                nc.vector.tensor_mul(t1[:], qkc[64:128, m, :], sin_s[64:128, :])
                nc.vector.tensor_sub(d0, t0[:], t1[:])
                t2 = p1s.tile([64, 512], F16, name="t2", tag="t0", bufs=1)
                nc.vector.tensor_mul(t2[:], qkc[64:128, m, :], cos_s[64:128, :])
                t3 = p1s.tile([64, 512], F16, name="t3", tag="t1", bufs=1)
                nc.vector.tensor_mul(t3[:], qkc[0:64, m, :], sin_s[0:64, :])
                nc.vector.tensor_add(d1, t2[:], t3[:])
            nc.vector.tensor_mul(g["vT_sb"][:, nb, :], qkc[:, QH + 1, :], rb[:])


def _phase15_vtok(nc, tc, g):
    """Transpose V to token-major ([tok,128] tiles) for the PV matmuls."""
    with tc.tile_pool(name="p15psum", bufs=1, space="PSUM") as p15p:
        for t in range(S // 128):
            nb, j = t // 4, t % 4
            tp = p15p.tile([128, 128], F16, name="tp", tag="tp", bufs=4)
            nc.tensor.transpose(tp[:], g["vT_sb"][:, nb, j * 128:(j + 1) * 128],
                                g["ident"][:])
            nc.vector.tensor_copy(g["v_tok"][:, t, :], tp[:])


def _phase2_attention(nc, tc, g, with_collectives, rg):
    with (
        tc.tile_pool(name="p2sbuf", bufs=2) as p2s,
        tc.tile_pool(name="p2psum", bufs=1, space="PSUM") as p2p,
    ):
        # prefetch phase-3 operands that have no dependency on attention:
        # the residual slice and the first 4 o_proj weight tiles
        nc.sync.dma_start(g["hsl"][:], g["hT_slice"][:, :, :])
        for m in range(4):
            nc.sync.dma_start(g["wo_pre"][:, m, :, :], g["wo"][:, m, :, :])

        for hh in range(QH):
            for qb in range(NB):
                qcols = slice(qb * 512, (qb + 1) * 512)
                npair = 2 * qb + 2
                att_ps = p2p.tile([128, 512], F32, name="att_ps", tag="att_ps", bufs=2)
                acc = p2s.tile([128, 2, 512], F16, name="acc", tag="acc", bufs=2)
                for pp in range(npair):
                    kt0, kt1 = 2 * pp, 2 * pp + 1
                    s_ps = p2p.tile([128, 2, 512], F32, name="s_ps", tag="s_ps", bufs=2)
                    nc.tensor.matmul(
                        s_ps[:, 0, :], g["kT_sb"][:, kt0 * 128:(kt0 + 1) * 128],
                        g["qT_sb"][:, hh, qcols], start=True, stop=True,
                    )
                    nc.tensor.matmul(
                        s_ps[:, 1, :], g["kT_sb"][:, kt1 * 128:(kt1 + 1) * 128],
                        g["qT_sb"][:, hh, qcols], start=True, stop=True,
                    )
                    e = p2s.tile([128, 2, 512], F16, name="e", tag="e", bufs=4)
                    nc.scalar.activation(e[:], s_ps[:], AF.Exp, scale=SCALE)
                    if pp >= 2 * qb:
                        j0 = 2 * (pp - 2 * qb)
                        nc.vector.tensor_mul(e[:], e[:], g["mask_sb"][:, j0:j0 + 2, :])
                    if pp == 0:
                        nc.vector.tensor_copy(acc[:], e[:])
                    else:
                        nc.vector.tensor_add(acc[:], acc[:], e[:])
                    nc.tensor.matmul(att_ps[:], g["v_tok"][:, kt0, :], e[:, 0, :],
                                     start=(pp == 0), stop=False)
                    nc.tensor.matmul(att_ps[:], g["v_tok"][:, kt1, :], e[:, 1, :],
                                     start=False, stop=(pp == npair - 1))
                sums_ps = p2p.tile([1, 512], F32, name="sums_ps", tag="sums_ps", bufs=2)
                nc.tensor.matmul(sums_ps[:], g["ones"][:], acc[:, 0, :],
                                 start=True, stop=False)
                nc.tensor.matmul(sums_ps[:], g["ones"][:], acc[:, 1, :],
                                 start=False, stop=True)
                recip = p2s.tile([1, 512], F32, name="recip", tag="recip")
                nc.vector.reciprocal(recip[:], sums_ps[:])
                recip16 = p2s.tile([1, 512], F16, name="recip16", tag="recip16")
                nc.vector.tensor_copy(recip16[:], recip[:])
                rb2 = p2s.tile([128, 512], F16, name="rb2", tag="rb2", bufs=2)
                nc.gpsimd.partition_broadcast(rb2[:], recip16[:])
                anorm = p2s.tile([128, 512], F16, name="anorm", tag="anorm", bufs=2)
                nc.vector.tensor_mul(anorm[:], att_ps[:], rb2[:])
                for half in range(2):
                    dst_core = qb * 2 + half
                    nc.sync.dma_start(
                        g[f"a2a_in{hh}"][dst_core * 128:(dst_core + 1) * 128, :],
                        anorm[:, half * 256:(half + 1) * 256],
                    )
            # ship this head's context while the next head computes
            if with_collectives:
                nc.gpsimd.collective_compute(
                    "AllToAll", mybir.AluOpType.bypass, replica_groups=rg,
                    ins=[g[f"a2a_in{hh}"].opt()], outs=[g[f"a2a_out{hh}"].opt()],
                )
            else:
                nc.sync.dma_start(g[f"a2a_out{hh}"][:, :], g[f"a2a_in{hh}"][:, :])
            # pull this head's context into the o_proj operand layout
            for r in range(NC):
                nc.sync.dma_start(
                    g["asl"][:, r * QH + hh, :],
                    g[f"a2a_out{hh}"][r * 128:(r + 1) * 128, :],
                )


def _phase3_oproj(nc, tc, g):
    with (
        tc.tile_pool(name="p3sbuf", bufs=2) as p3s,
        tc.tile_pool(name="p3psum", bufs=1, space="PSUM") as p3p,
    ):
        res2 = g["res2"]
        st2_ps = p3p.tile([1, TPC], F32, name="st2_ps", tag="st2_ps")
        for m in range(KH):
            if m < 4:
                wsrc = g["wo_pre"][:, m]
            else:
                wob = p3s.tile([128, KH, 128], F16, name="wob", tag="wob", bufs=3)
                nc.sync.dma_start(wob[:], g["wo"][:, m, :, :])
                wsrc = wob
            o_ps = p3p.tile([128, TPC], F32, name="o_ps", tag="o_ps", bufs=2)
            for k in range(KH):
                nc.tensor.matmul(o_ps[:], wsrc[:, k, :], g["asl"][:, k, :],
                                 start=(k == 0), stop=(k == KH - 1))
            nc.vector.tensor_add(res2[:, m, :], o_ps[:], g["hsl"][:, m, :])
            nc.sync.dma_start(g["res_out"][m * 128:(m + 1) * 128, :], res2[:, m, :])
            sq2 = p3s.tile([128, TPC], F16, name="sq2", tag="sq2", bufs=2)
            nc.vector.tensor_mul(sq2[:], res2[:, m, :], res2[:, m, :])
            nc.tensor.matmul(st2_ps[:], g["ones"][:], sq2[:],
                             start=(m == 0), stop=(m == KH - 1))
        std2 = p3s.tile([1, TPC], F32, name="std2", tag="std2")
        nc.scalar.activation(std2[:], st2_ps[:], AF.Sqrt, bias=g["epsb"][:], scale=1.0 / H)
        rstd2 = p3s.tile([1, TPC], F32, name="rstd2", tag="rstd2")
        nc.vector.reciprocal(rstd2[:], std2[:])
        rstd2_16 = p3s.tile([1, TPC], F16, name="rstd2_16", tag="rstd2_16")
        nc.vector.tensor_copy(rstd2_16[:], rstd2[:])
        rb3 = p3s.tile([128, TPC], F16, name="rb3", tag="rb3")
        nc.gpsimd.partition_broadcast(rb3[:], rstd2_16[:])
        for m in range(KH):
            x2 = p3s.tile([128, TPC], F16, name="x2", tag="x2", bufs=2)
            nc.vector.tensor_mul(x2[:], res2[:, m, :], rb3[:])
            nc.sync.dma_start(g["ag2_in"][:, m, :], x2[:])


def _phase4_gate_up(nc, tc, g):
    with (
        tc.tile_pool(name="p4big", bufs=1) as p4b,
        tc.tile_pool(name="p4sbuf", bufs=2) as p4s,
        tc.tile_pool(name="p4psum", bufs=1, space="PSUM") as p4p,
    ):
        x2h = p4b.tile([128, KH, S], F16, name="x2h")  # 16.8 MB
        for r in range(NC):
            for kq in range(4):
                nc.sync.dma_start(
                    x2h[:, kq * 8:(kq + 1) * 8, r * 256:(r + 1) * 256],
                    g["ag2_out"][r * 128:(r + 1) * 128, kq * 8:(kq + 1) * 8, :],
                )
        for m in range(MB_GU):
            gu = p4s.tile([128, KH, 256], F16, name="gu", tag="gu", bufs=2)
            nc.sync.dma_start(gu[:], g["wgu"][:, m, :, :])
            for tb in range(NB):
                tcols = slice(tb * 512, (tb + 1) * 512)
                g_ps = p4p.tile([128, 512], F32, name="g_ps", tag="g_ps", bufs=2)
                for k in range(KH):
                    nc.tensor.matmul(g_ps[:], gu[:, k, 0:128], x2h[:, k, tcols],
                                     start=(k == 0), stop=(k == KH - 1))
                u_ps = p4p.tile([128, 512], F32, name="u_ps", tag="u_ps", bufs=2)
                for k in range(KH):
                    nc.tensor.matmul(u_ps[:], gu[:, k, 128:256], x2h[:, k, tcols],
                                     start=(k == 0), stop=(k == KH - 1))
                sg = p4s.tile([128, 512], F16, name="sg", tag="sg", bufs=2)
                nc.scalar.activation(sg[:], g_ps[:], AF.Silu)
                hhh = p4s.tile([128, 512], F16, name="hhh", tag="hhh", bufs=2)
                nc.vector.tensor_mul(hhh[:], sg[:], u_ps[:])
                nc.sync.dma_start(g["h_dram"][:, m, tcols], hhh[:])


def _phase5_down(nc, tc, g, with_collectives, rg):
    with (
        tc.tile_pool(name="p5big", bufs=1) as p5b,
        tc.tile_pool(name="p5sbuf", bufs=2) as p5s,
        tc.tile_pool(name="p5psum", bufs=1, space="PSUM") as p5p,
    ):
        # chunked per k so each load fires as soon as phase 4 has written
        # that k-tile of h — by phase-4 end most of hful is already resident
        hful = p5b.tile([128, KI, S], F16, name="hful")  # 7.3 MB
        for k in range(KI):
            nc.sync.dma_start(hful[:, k, :], g["h_dram"][:, k, :])
        for r in range(8):
            for mi in range(KH // 8):
                m = r * (KH // 8) + mi
                db = p5s.tile([128, KI, 128], F16, name="db", tag="db", bufs=3)
                nc.sync.dma_start(db[:], g["wdn"][:, m, :, :])
                for tb in range(NB):
                    tcols = slice(tb * 512, (tb + 1) * 512)
                    d_ps = p5p.tile([128, 512], F32, name="d_ps", tag="d_ps", bufs=2)
                    for k in range(KI):
                        nc.tensor.matmul(d_ps[:], db[:, k, :], hful[:, k, tcols],
                                         start=(k == 0), stop=(k == KI - 1))
                    ot = p5s.tile([128, 512], F16, name="ot", tag="ot", bufs=2)
                    nc.vector.tensor_copy(ot[:], d_ps[:])
                    nc.sync.dma_start(g[f"rs_in{r}"][mi * 128:(mi + 1) * 128, tcols], ot[:])
            if with_collectives:
                nc.gpsimd.collective_compute(
                    "ReduceScatter", mybir.AluOpType.add, replica_groups=rg,
                    ins=[g[f"rs_in{r}"].opt()], outs=[g[f"rs_out{r}"].opt()],
                )
            else:
                nc.sync.dma_start(g[f"rs_out{r}"][:, :], g[f"rs_in{r}"][0:H // NC // 8, :])
            nc.sync.dma_start(
                g["out_down"][r * 64:(r + 1) * 64, :], g[f"rs_out{r}"][:, :])


def build_program(with_collectives=True, stop_after=99):
    nc = bacc.Bacc("TRN2", target_bir_lowering=False, debug=False, num_devices=NC)

    g = {}
    g["hT"] = nc.dram_tensor("hT", [H, S], F16, kind="ExternalInput")
    g["hT_slice"] = nc.dram_tensor("hT_slice", [128, KH, TPC], F16, kind="ExternalInput")
    g["wqkv"] = nc.dram_tensor("wqkv", [128, KH, (QH + 2) * 128], F16, kind="ExternalInput")
    g["wo"] = nc.dram_tensor("wo", [128, KH, KH, 128], F16, kind="ExternalInput")
    g["wgu"] = nc.dram_tensor("wgu", [128, MB_GU, KH, 256], F16, kind="ExternalInput")
    g["wdn"] = nc.dram_tensor("wdn", [128, KH, KI, 128], F16, kind="ExternalInput")
    g["cosT"] = nc.dram_tensor("cosT", [128, S], F16, kind="ExternalInput")
    g["sinT"] = nc.dram_tensor("sinT", [128, S], F16, kind="ExternalInput")
    g["masks"] = nc.dram_tensor("masks", [128, 4, 512], F16, kind="ExternalInput")

    g["res_out"] = nc.dram_tensor("res_out", [H, TPC], F16, kind="ExternalOutput")
    g["out_down"] = nc.dram_tensor("out_down", [H // NC, S], F16, kind="ExternalOutput")

    rg = [list(range(NC))]

    with tile.TileContext(nc) as tc:
        with (
            tc.tile_pool(name="consts", bufs=1) as consts,
            tc.tile_pool(name="dram", bufs=1, space="DRAM") as dram,
        ):
            for hh in range(QH):
                g[f"a2a_in{hh}"] = dram.tile([NC * 128, TPC], F16, name=f"a2a_in{hh}")
                g[f"a2a_out{hh}"] = dram.tile([NC * 128, TPC], F16, name=f"a2a_out{hh}")
            g["ag2_in"] = dram.tile([128, KH, TPC], F16, name="ag2_in")
            g["ag2_out"] = dram.tile([NC * 128, KH, TPC], F16, name="ag2_out", addr_space="Shared")
            g["h_dram"] = dram.tile([128, KI, S], F16, name="h_dram")
            for r in range(8):
                g[f"rs_in{r}"] = dram.tile([H // 8, S], F16, name=f"rs_in{r}")
                g[f"rs_out{r}"] = dram.tile([H // NC // 8, S], F16, name=f"rs_out{r}")

            ones32 = consts.tile([128, 1], F32, name="ones32")
            nc.gpsimd.memset(ones32[:], 1.0)
            g["ones"] = consts.tile([128, 1], F16, name="ones")
            nc.vector.tensor_copy(g["ones"][:], ones32[:])
            ident32 = consts.tile([128, 128], F32, name="ident32")
            make_identity(nc, ident32[:])
            g["ident"] = consts.tile([128, 128], F16, name="ident")
            nc.vector.tensor_copy(g["ident"][:], ident32[:])
            g["epsb"] = consts.tile([1, 1], F32, name="epsb")
            nc.gpsimd.memset(g["epsb"][:], EPS)

            # phase-3 operands that outlive the attention pools
            with tc.tile_pool(name="p3keep", bufs=1) as p3keep:
                g["hsl"] = p3keep.tile([128, KH, TPC], F16, name="hsl")       # 2 MB
                g["wo_pre"] = p3keep.tile([128, 4, KH, 128], F16, name="wo_pre")  # 4 MB
                g["asl"] = p3keep.tile([128, KH, TPC], F16, name="asl")       # 2 MB
                g["res2"] = p3keep.tile([128, KH, TPC], F16, name="res2")     # 2 MB

                with tc.tile_pool(name="attn", bufs=1) as attn:
                    g["cos_sb"] = attn.tile([128, S], F16, name="cos_sb")
                    g["sin_sb"] = attn.tile([128, S], F16, name="sin_sb")
                    g["mask_sb"] = attn.tile([128, 4, 512], F16, name="mask_sb")
                    g["qT_sb"] = attn.tile([128, QH, S], F16, name="qT_sb")          # 2 MB
                    g["kT_sb"] = attn.tile([128, S], F16, name="kT_sb")              # 0.5 MB
                    g["vT_sb"] = attn.tile([128, NB, 512], F16, name="vT_sb")        # 0.5 MB
                    g["v_tok"] = attn.tile([128, S // 128, 128], F16, name="v_tok")  # 0.5 MB

                    _phase1_qkv(nc, tc, g)
                    if stop_after >= 2:
                        _phase15_vtok(nc, tc, g)
                        _phase2_attention(nc, tc, g, with_collectives, rg)

                if stop_after >= 3:
                    _phase3_oproj(nc, tc, g)
                    if with_collectives:
                        nc.gpsimd.collective_compute(
                            "AllGather", mybir.AluOpType.bypass, replica_groups=rg,
                            ins=[g["ag2_in"].opt()], outs=[g["ag2_out"].opt()],
                        )
                    else:
                        nc.sync.dma_start(g["ag2_out"][0:128, :, :], g["ag2_in"][:, :, :])

            if stop_after >= 4:
                _phase4_gate_up(nc, tc, g)

            if stop_after >= 5:
                _phase5_down(nc, tc, g, with_collectives, rg)

    nc.finalize()
    return nc


_cached_nc = None


def _get_nc():
    global _cached_nc
    if _cached_nc is None:
        _cached_nc = build_program(with_collectives=True)
    return _cached_nc


def _host_prep(positions, hidden_states, w_qkv, w_o, w_gate_up, w_down, ln1_w, ln2_w):
    f32 = np.float32
    f16 = np.float16
    hidden = np.asarray(hidden_states, dtype=f32)[0]          # [S, H]
    hT = np.ascontiguousarray(hidden.T).astype(f16)            # [H, S]
    pos = np.asarray(positions).astype(f32)[0]                 # [S]

    half = HD // 2
    inv_freq = (1.0 / (f32(THETA) ** (np.arange(0, half, dtype=f32) / f32(half)))).astype(f32)
    ang = pos[:, None] * inv_freq[None, :]                     # [S, 64] fp32
    cos_half = np.cos(ang).astype(f32).T                       # [64, S]
    sin_half = np.sin(ang).astype(f32).T
    cosT_np = np.concatenate([cos_half, cos_half], axis=0).astype(f16)  # [128, S]
    sinT_np = np.concatenate([sin_half, sin_half], axis=0).astype(f16)

    w_qkv_f = np.asarray(w_qkv, dtype=f32) * np.asarray(ln1_w, dtype=f32)[:, None]
    w_gu_f = np.asarray(w_gate_up, dtype=f32) * np.asarray(ln2_w, dtype=f32)[:, None]
    w_o_f = np.ascontiguousarray(
        np.asarray(w_o, dtype=f32).reshape(KH, 128, KH, 128).transpose(1, 2, 0, 3)
    ).astype(f16)
    w_dn_f = np.asarray(w_down, dtype=f32)

    kk = np.arange(128)[:, None, None]
    jj = np.arange(4)[None, :, None]
    qq = np.arange(512)[None, None, :]
    masks_np = np.ascontiguousarray((qq >= kk + 128 * jj).astype(f16))  # [128, 4, 512]

    in_maps = []
    for c in range(NC):
        q_cols = w_qkv_f[:, c * QH * HD:(c + 1) * QH * HD]
        k_col = w_qkv_f[:, NQ * HD + c * HD: NQ * HD + (c + 1) * HD]
        v_col = w_qkv_f[:, (NQ + NKV) * HD + c * HD: (NQ + NKV) * HD + (c + 1) * HD]
        wqkv_c = np.concatenate([q_cols, k_col, v_col], axis=1)
        wqkv_c = np.ascontiguousarray(
            wqkv_c.reshape(KH, 128, (QH + 2) * 128).transpose(1, 0, 2)).astype(f16)
        # per-m interleave: [128, m, k, gate128|up128]
        wg_c = w_gu_f[:, c * IPC:(c + 1) * IPC].reshape(KH, 128, MB_GU, 128)
        wu_c = w_gu_f[:, I + c * IPC: I + (c + 1) * IPC].reshape(KH, 128, MB_GU, 128)
        wgu_c = np.ascontiguousarray(
            np.concatenate([wg_c[..., None, :], wu_c[..., None, :]], axis=3)
            .reshape(KH, 128, MB_GU, 256).transpose(1, 2, 0, 3)).astype(f16)
        wdn_c = np.ascontiguousarray(
            w_dn_f[c * IPC:(c + 1) * IPC, :].reshape(KI, 128, KH, 128)
            .transpose(1, 2, 0, 3)).astype(f16)
        hT_slice_c = np.ascontiguousarray(
            hT[:, c * TPC:(c + 1) * TPC].reshape(KH, 128, TPC).transpose(1, 0, 2))
        in_maps.append({
            "hT": hT,
            "hT_slice": hT_slice_c,
            "wqkv": wqkv_c,
            "wo": w_o_f,
            "wgu": wgu_c,
            "wdn": wdn_c,
            "cosT": cosT_np,
            "sinT": sinT_np,
            "masks": masks_np,
        })
    return in_maps


def kernel(**inputs):
    in_maps = _host_prep(**inputs)
    nc = _get_nc()
    res = run_bass_kernel_spmd(nc, in_maps, core_ids=list(range(NC)))
    results = res.results

    outT = np.empty((H, S), np.float32)
    for c in range(NC):
        od = results[c]["out_down"].astype(np.float32)  # [512, S]
        for r in range(8):
            outT[512 * r + 64 * c: 512 * r + 64 * (c + 1)] = od[64 * r:64 * (r + 1)]
    resT = np.concatenate(
        [results[c]["res_out"].astype(np.float32) for c in range(NC)], axis=1)  # [H, S]
    out = np.ascontiguousarray(outT.T).reshape(1, S, H).astype(np.float32)
    residual = np.ascontiguousarray(resT.T).reshape(1, S, H).astype(np.float32)
    return out, residual


# revision 53
# speedup vs baseline: 1.0669x; 1.0669x over previous
"""Bamba attention decoder layer on 8 Trainium2 NeuronCores.

Sharding: tensor-parallel attention (4 q heads + 1 kv head per core),
AllToAll of attention context (delivers each core its token slice at a static
address), token-sliced o_proj + fused add/rmsnorm, AllGather of normed
activations, I-sharded SwiGLU MLP (1792 cols/core), ReduceScatter of
down-proj partials.

Layout: feature-major activations ([features->partitions, tokens->free]) so
every linear layer uses its natural-layout weight block as the stationary
matmul operand. All activations/weights are fp16 (psum accumulation stays
fp32): halves DMA traffic and doubles DVE element throughput at unchanged PE
rate. The 1/rms factor of rmsnorm1 is applied after the QKV matmul (per-token
column scaling commutes through the contraction); ln weights are folded into
the weight matrices on the host.

Schedule notes (vs the original fp32 version):
- qkv weight chunks are interleaved with the first token block's hb loads so
  the PE starts ~5us in instead of waiting for the full 12.6MB.
- all 6 qkv psum banks are evacuated by immediate scalar-engine copies; the
  rmsnorm/rope chain runs from SBUF off the PE critical path, and the V
  transposes are deferred to a mini-phase after the last block.
- attention processes score tiles in pairs ([128,1024] psum tiles) to halve
  exp/mask/accumulate instruction count; probs in fp16.
- o_proj weights (fp16) stream under the matmul; the first 4 m-tiles and the
  residual slice prefetch during attention.
- the MLP first matmul is single-pass (all 2048 tokens resident in fp16), so
  gate/up weights load once; h round-trips DRAM in fp16 with the phase-5
  reload chunked per k-tile so it overlaps phase 4.
"""

import numpy as np

import concourse.bacc as bacc
import concourse.mybir as mybir
import concourse.tile as tile
from concourse.bass_utils import run_bass_kernel_spmd
from concourse.masks import make_identity

NC = 8
S = 2048
H = 4096
HD = 128
NQ = 32
NKV = 8
I = 14336
QH = NQ // NC        # q heads per core = 4
IPC = I // NC        # intermediate cols per core = 1792
TPC = S // NC        # tokens per core = 256
EPS = 1e-5
THETA = 10000.0
SCALE = HD ** -0.5

F32 = mybir.dt.float32
F16 = mybir.dt.float16

KH = H // 128        # 32 k-tiles over H
NB = S // 512        # 4 token blocks of 512
MB_GU = IPC // 128   # 14 m tiles for gate (and for up)
KI = IPC // 128      # 14 k tiles over I per core

AF = mybir.ActivationFunctionType


def _phase1_qkv(nc, tc, g):
    """QKV matmul + rmsnorm1 stats + rope. Fills qT_sb/kT_sb/vT_sb."""
    with (
        tc.tile_pool(name="p1sbuf", bufs=2) as p1s,
        tc.tile_pool(name="p1psum", bufs=1, space="PSUM") as p1p,
    ):
        for nb in range(NB):
            ncols = slice(nb * 512, (nb + 1) * 512)
            st_ps = p1p.tile([1, 512], F32, name="st_ps", tag="st_ps")
            # one psum tile per output block so dependency tracking stays
            # per-bank: the next token block's matmul for slot m waits only
            # on slot m's evacuation copy
            mm_ps = [p1p.tile([128, 512], F32, name=f"mm_ps{m}", tag=f"mm_ps{m}")
                     for m in range(QH + 2)]
            for k in range(KH):
                # weights stream alongside the activations (re-read per
                # block; DMA has slack and this keeps SBUF residency low)
                wqk = p1s.tile([128, (QH + 2) * 128], F16, name="wqk",
                               tag="wqk", bufs=6)
                nc.sync.dma_start(wqk[:], g["wqkv"][:, k, :])
                hb = p1s.tile([128, 512], F16, name="hb", tag="hb", bufs=5)
                nc.sync.dma_start(hb[:], g["hT"][k * 128:(k + 1) * 128, ncols])
                sq = p1s.tile([128, 512], F16, name="sq", tag="sq", bufs=3)
                nc.scalar.activation(sq[:], hb[:], AF.Square)
                nc.tensor.matmul(st_ps[:], g["ones"][:], sq[:],
                                 start=(k == 0), stop=(k == KH - 1))
                for m in range(QH + 2):
                    nc.tensor.matmul(
                        mm_ps[m][:], wqk[:, m * 128:(m + 1) * 128], hb[:],
                        start=(k == 0), stop=(k == KH - 1),
                    )
                if k == 20:
                    # rope tables for this block, needed right after the
                    # k-loop (spread across blocks to keep DMA bursts small)
                    nc.sync.dma_start(g["cos_sb"][:, ncols], g["cosT"][:, ncols])
                    nc.sync.dma_start(g["sin_sb"][:, ncols], g["sinT"][:, ncols])
                    if nb == 0:
                        nc.sync.dma_start(g["mask_sb"][:], g["masks"][:, :])
            # evacuate all 6 psum banks immediately (scalar engine) so the
            # next block's matmuls only wait on these copies, not on the
            # rmsnorm/rope chain below
            qkc = p1s.tile([128, QH + 2, 512], F16, name="qkc", tag="qkc", bufs=1)
            for m in range(QH + 2):
                nc.vector.tensor_copy(qkc[:, m, :], mm_ps[m][:])
            std_row = p1s.tile([1, 512], F32, name="std_row", tag="std_row")
            nc.scalar.activation(std_row[:], st_ps[:], AF.Sqrt,
                                 bias=g["epsb"][:], scale=1.0 / H)
            rstd_row = p1s.tile([1, 512], F32, name="rstd_row", tag="rstd_row")
            nc.vector.reciprocal(rstd_row[:], std_row[:])
            rstd16 = p1s.tile([1, 512], F16, name="rstd16", tag="rstd16")
            nc.vector.tensor_copy(rstd16[:], rstd_row[:])
            rb = p1s.tile([128, 512], F16, name="rb", tag="rb", bufs=1)
            nc.gpsimd.partition_broadcast(rb[:], rstd16[:])
            cos_s = p1s.tile([128, 512], F16, name="cos_s", tag="cos_s", bufs=1)
            nc.vector.tensor_mul(cos_s[:], g["cos_sb"][:, ncols], rb[:])
            sin_s = p1s.tile([128, 512], F16, name="sin_s", tag="sin_s", bufs=1)
            nc.vector.tensor_mul(sin_s[:], g["sin_sb"][:, ncols], rb[:])
            for m in range(QH + 1):
                if m < QH:
                    d0 = g["qT_sb"][0:64, m, ncols]
                    d1 = g["qT_sb"][64:128, m, ncols]
                else:
                    d0 = g["kT_sb"][0:64, ncols]
                    d1 = g["kT_sb"][64:128, ncols]
                t0 = p1s.tile([64, 512], F16, name="t0", tag="t0", bufs=1)
                nc.vector.tensor_mul(t0[:], qkc[0:64, m, :], cos_s[0:64, :])
                t1 = p1s.tile([64, 512], F16, name="t1", tag="t1", bufs=1)
                nc.vector.tensor_mul(t1[:], qkc[64:128, m, :], sin_s[64:128, :])
                nc.vector.tensor_sub(d0, t0[:], t1[:])
                t2 = p1s.tile([64, 512], F16, name="t2", tag="t0", bufs=1)
                nc.vector.tensor_mul(t2[:], qkc[64:128, m, :], cos_s[64:128, :])
                t3 = p1s.tile([64, 512], F16, name="t3", tag="t1", bufs=1)
                nc.vector.tensor_mul(t3[:], qkc[0:64, m, :], sin_s[0:64, :])
                nc.vector.tensor_add(d1, t2[:], t3[:])
            nc.vector.tensor_mul(g[f"vT{nb}"][:, :], qkc[:, QH + 1, :], rb[:])

        # V transposes for blocks 0-1 (later blocks are deferred into phase 2
        # so their rmsnorm chains don't stall the PE between phase 1 and 2)
        tp1 = p1p.tile([128, 8, 128], F16, name="tp1", tag="tp1")
        for t in range(8):
            _vtok_one(nc, tp1, g, t)


def _vtok_one(nc, tp, g, t):
    """Transpose one 128-token V tile to token-major for the PV matmuls."""
    nb, j = t // 4, t % 4
    nc.tensor.transpose(tp[:, t % 8, :], g[f"vT{nb}"][:, j * 128:(j + 1) * 128],
                        g["ident"][:])
    nc.vector.tensor_copy(g["v_tok"][:, t, :], tp[:, t % 8, :])


def _phase2_attention(nc, tc, g, with_collectives, rg):
    with (
        tc.tile_pool(name="p2sbuf", bufs=2) as p2s,
        tc.tile_pool(name="p2psum", bufs=1, space="PSUM") as p2p,
    ):
        # prefetch phase-3/4/5 operands with no dependency on attention
        # (phase 2 has plenty of DMA slack)
        nc.sync.dma_start(g["hsl"][:], g["hT_slice"][:, :, :])
        for m in range(3):
            nc.sync.dma_start(g["wo_pre"][:, m, :, :], g["wo"][:, m, :, :])
        nc.sync.dma_start(g["gu_pre"][:], g["wgu"][:, 0, :, :])
        nc.sync.dma_start(g["wdn_pre"][:], g["wdn"][:, 0, :, :])

        for hh in range(QH):
            anob = p2s.tile([128, NB, 512], F16, name="anob", tag="anob", bufs=2)
            for qb in range(NB):
                if hh == 0 and qb == 1:
                    # blocks 2-3's V transposes: emitted after the first score
                    # block so the PE never waits on their rmsnorm chains
                    tp2 = p2p.tile([128, 8, 128], F16, name="tp2", tag="tp2")
                    for t in range(8, 16):
                        _vtok_one(nc, tp2, g, t)
                qcols = slice(qb * 512, (qb + 1) * 512)
                att_ps = p2p.tile([128, 512], F32, name="att_ps", tag="att_ps", bufs=2)
                acc = p2s.tile([128, 2, 512], F16, name="acc", tag="acc", bufs=2)
                # full (unmasked) score tiles, processed in pairs
                for pp in range(2 * qb):
                    kt0, kt1 = 2 * pp, 2 * pp + 1
                    s_ps = p2p.tile([128, 2, 512], F32, name="s_ps", tag="s_ps", bufs=2)
                    nc.tensor.matmul(
                        s_ps[:, 0, :], g["kT_sb"][:, kt0 * 128:(kt0 + 1) * 128],
                        g["qT_sb"][:, hh, qcols], start=True, stop=True,
                    )
                    nc.tensor.matmul(
                        s_ps[:, 1, :], g["kT_sb"][:, kt1 * 128:(kt1 + 1) * 128],
                        g["qT_sb"][:, hh, qcols], start=True, stop=True,
                    )
                    e = p2s.tile([128, 2, 512], F16, name="e", tag="e", bufs=4)
                    nc.scalar.activation(e[:], s_ps[:], AF.Exp, scale=SCALE)
                    if pp == 0:
                        nc.vector.tensor_copy(acc[:], e[:])
                    else:
                        nc.vector.tensor_add(acc[:], acc[:], e[:])
                    nc.tensor.matmul(att_ps[:], g["v_tok"][:, kt0, :], e[:, 0, :],
                                     start=(pp == 0), stop=False)
                    nc.tensor.matmul(att_ps[:], g["v_tok"][:, kt1, :], e[:, 1, :],
                                     start=False, stop=False)
                # the 4 diagonal tiles: causality restricts tile j to local
                # q >= 128j, so matmul/exp/accumulate only the valid width and
                # apply a 128x128 triangle mask to the leading sub-block
                for pj in range(2):
                    s_ps = p2p.tile([128, 2, 512], F32, name="s_ps", tag="s_ps", bufs=2)
                    e = p2s.tile([128, 2, 512], F16, name="e", tag="e", bufs=4)
                    for i in range(2):
                        j = 2 * pj + i
                        kt = 4 * qb + j
                        qoff, w = 128 * j, 512 - 128 * j
                        nc.tensor.matmul(
                            s_ps[:, i, 0:w], g["kT_sb"][:, kt * 128:(kt + 1) * 128],
                            g["qT_sb"][:, hh, qb * 512 + qoff:(qb + 1) * 512],
                            start=True, stop=True,
                        )
                    if pj == 0:
                        # j=0 is full width; j=1 region [384:512) holds stale
                        # psum, exp'd but never read
                        nc.scalar.activation(e[:], s_ps[:], AF.Exp, scale=SCALE)
                    else:
                        nc.scalar.activation(e[:, 0, 0:256], s_ps[:, 0, 0:256],
                                             AF.Exp, scale=SCALE)
                        nc.scalar.activation(e[:, 1, 0:128], s_ps[:, 1, 0:128],
                                             AF.Exp, scale=SCALE)
                    for i in range(2):
                        j = 2 * pj + i
                        kt = 4 * qb + j
                        qoff, w = 128 * j, 512 - 128 * j
                        nc.vector.tensor_mul(e[:, i, 0:128], e[:, i, 0:128],
                                             g["mask_sb"][:, :])
                        if qb == 0 and j == 0:
                            nc.vector.tensor_copy(acc[:, 0, :], e[:, 0, :])
                        elif qb == 0 and j == 1:
                            nc.gpsimd.memset(acc[:, 1, 0:128], 0.0)
                            nc.vector.tensor_copy(acc[:, 1, 128:512], e[:, 1, 0:384])
                        else:
                            nc.vector.tensor_add(acc[:, i, qoff:512],
                                                 acc[:, i, qoff:512], e[:, i, 0:w])
                        nc.tensor.matmul(
                            att_ps[:, qoff:512], g["v_tok"][:, kt, :], e[:, i, 0:w],
                            start=(qb == 0 and j == 0), stop=(j == 3),
                        )
                sums_ps = p2p.tile([1, 512], F32, name="sums_ps", tag="sums_ps", bufs=1)
                nc.tensor.matmul(sums_ps[:], g["ones"][:], acc[:, 0, :],
                                 start=True, stop=False)
                nc.tensor.matmul(sums_ps[:], g["ones"][:], acc[:, 1, :],
                                 start=False, stop=True)
                recip = p2s.tile([1, 512], F32, name="recip", tag="recip")
                nc.vector.reciprocal(recip[:], sums_ps[:])
                recip16 = p2s.tile([1, 512], F16, name="recip16", tag="recip16")
                nc.vector.tensor_copy(recip16[:], recip[:])
                rb2 = p2s.tile([128, 512], F16, name="rb2", tag="rb2", bufs=2)
                nc.gpsimd.partition_broadcast(rb2[:], recip16[:])
                nc.vector.tensor_mul(anob[:, qb, :], att_ps[:], rb2[:])
            # one batched DMA scatters this head's context into the A2A
            # input layout ([core, 128, 256]); per-partition runs stay 512B
            nc.sync.dma_start(
                g[f"a2a_in{hh}"][:, :, :].transpose([1, 0, 2]),
                anob[:, :, :],
            )
            # ship this head's context while the next head computes
            if with_collectives:
                nc.gpsimd.collective_compute(
                    "AllToAll", mybir.AluOpType.bypass, replica_groups=rg,
                    ins=[g[f"a2a_in{hh}"].opt()], outs=[g[f"a2a_out{hh}"].opt()],
                )
                # pull this head's context into the o_proj operand layout
                nc.sync.dma_start(
                    g["asl"][:, hh, :, :],
                    g[f"a2a_out{hh}"][:, :, :].transpose([1, 0, 2]),
                )
            else:
                # local stand-in: same byte count, sourced from SBUF so the
                # o_proj operand isn't chained behind the a2a_in write
                nc.sync.dma_start(g["asl"][:, hh, :, :], anob[:, :, :])


def _phase3_oproj(nc, tc, g, with_collectives, rg):
    with (
        tc.tile_pool(name="p3sbuf", bufs=2) as p3s,
        tc.tile_pool(name="p3psum", bufs=1, space="PSUM") as p3p,
    ):
        res2 = p3s.tile([128, KH, TPC], F16, name="res2", tag="res2", bufs=1)
        st2_ps = p3p.tile([1, TPC], F32, name="st2_ps", tag="st2_ps")
        asl = g["asl"]
        for m in range(KH):
            if m < 3:
                wsrc = g["wo_pre"][:, m]
            else:
                wob = p3s.tile([128, KH, 128], F16, name="wob", tag="wob", bufs=3)
                nc.sync.dma_start(wob[:], g["wo"][:, m, :, :])
                wsrc = wob
            o_ps = p3p.tile([128, TPC], F32, name="o_ps", tag="o_ps", bufs=2)
            for k in range(KH):
                nc.tensor.matmul(o_ps[:], wsrc[:, k, :], asl[:, k // 8, k % 8, :],
                                 start=(k == 0), stop=(k == KH - 1))
            nc.vector.tensor_add(res2[:, m, :], o_ps[:], g["hsl"][:, m, :])
            nc.sync.dma_start(g["res_out"][m * 128:(m + 1) * 128, :], res2[:, m, :])
            sq2 = p3s.tile([128, TPC], F16, name="sq2", tag="sq2", bufs=2)
            nc.vector.tensor_mul(sq2[:], res2[:, m, :], res2[:, m, :])
            nc.tensor.matmul(st2_ps[:], g["ones"][:], sq2[:],
                             start=(m == 0), stop=(m == KH - 1))
        std2 = p3s.tile([1, TPC], F32, name="std2", tag="std2")
        nc.scalar.activation(std2[:], st2_ps[:], AF.Sqrt, bias=g["epsb"][:], scale=1.0 / H)
        rstd2 = p3s.tile([1, TPC], F32, name="rstd2", tag="rstd2")
        nc.vector.reciprocal(rstd2[:], std2[:])
        rstd2_16 = p3s.tile([1, TPC], F16, name="rstd2_16", tag="rstd2_16")
        nc.vector.tensor_copy(rstd2_16[:], rstd2[:])
        rb3 = p3s.tile([128, TPC], F16, name="rb3", tag="rb3")
        nc.gpsimd.partition_broadcast(rb3[:], rstd2_16[:])
        for kq in range(4):
            # batch 8 feature-tiles of normed activations into one DMA, then
            # ship the quarter so phase 4's operands stream in incrementally
            x2b = p3s.tile([128, 8, TPC], F16, name="x2b", tag="x2b", bufs=2)
            for mi in range(8):
                nc.vector.tensor_mul(x2b[:, mi, :], res2[:, kq * 8 + mi, :], rb3[:])
            nc.sync.dma_start(g[f"ag2_in_q{kq}"][:, :, :], x2b[:, :, :])
            if with_collectives:
                nc.gpsimd.collective_compute(
                    "AllGather", mybir.AluOpType.bypass, replica_groups=rg,
                    ins=[g[f"ag2_in_q{kq}"].opt()],
                    outs=[g[f"ag2_out_q{kq}"].opt()],
                )
            else:
                # local stand-in for the gather: same bytes moved as the
                # DRAM-to-DRAM copy, but sourced from SBUF so phase 4's loads
                # aren't chained behind the ag2_in write
                nc.sync.dma_start(g[f"ag2_out_q{kq}"][0, :, :, :], x2b[:, :, :])


def _phase4_gate_up(nc, tc, g):
    """SwiGLU first half, one 512-token quarter at a time.

    Each quarter's activations (4.2MB fp16) double-buffer against the previous
    quarter's compute; gate/up weights re-stream per quarter (DMA has slack).
    h = silu(gate)*up is written straight to the SBUF-resident hful tile, so
    phase 5 starts with everything already on-chip.
    """
    with (
        tc.tile_pool(name="p4x", bufs=1) as p4x,
        tc.tile_pool(name="p4sbuf", bufs=2) as p4s,
        tc.tile_pool(name="p4psum", bufs=1, space="PSUM") as p4p,
    ):
        for tb in range(NB):
            tcols = slice(tb * 512, (tb + 1) * 512)
            x2q = p4x.tile([128, KH, 512], F16, name="x2q", tag="x2q", bufs=2)
            for kq in range(4):
                # one DMA per feature-quarter (both source cores adjacent in
                # the gather buffer) so the k-loop can start on quarter 0
                nc.sync.dma_start(
                    x2q[:, kq * 8:(kq + 1) * 8, :],
                    g[f"ag2_out_q{kq}"][2 * tb:2 * tb + 2, :, :, :]
                    .transpose([1, 2, 0, 3]),
                )
            for m in range(MB_GU):
                if m == 0:
                    gu = g["gu_pre"]
                else:
                    gu = p4s.tile([128, KH, 256], F16, name="gu", tag="gu", bufs=2)
                    nc.sync.dma_start(gu[:], g["wgu"][:, m, :, :])
                g_ps = p4p.tile([128, 512], F32, name="g_ps", tag="g_ps", bufs=2)
                for k in range(KH):
                    nc.tensor.matmul(g_ps[:], gu[:, k, 0:128], x2q[:, k, :],
                                     start=(k == 0), stop=(k == KH - 1))
                u_ps = p4p.tile([128, 512], F32, name="u_ps", tag="u_ps", bufs=2)
                for k in range(KH):
                    nc.tensor.matmul(u_ps[:], gu[:, k, 128:256], x2q[:, k, :],
                                     start=(k == 0), stop=(k == KH - 1))
                sg = p4s.tile([128, 512], F16, name="sg", tag="sg", bufs=2)
                nc.scalar.activation(sg[:], g_ps[:], AF.Silu)
                nc.vector.tensor_mul(g["hful"][:, m, tcols], sg[:], u_ps[:])


def _phase5_down(nc, tc, g, with_collectives, rg):
    with (
        tc.tile_pool(name="p5sbuf", bufs=2) as p5s,
        tc.tile_pool(name="p5psum", bufs=1, space="PSUM") as p5p,
    ):
        hful = g["hful"]
        for r in range(8):
            for mi in range(KH // 8):
                m = r * (KH // 8) + mi
                if m == 0:
                    db = g["wdn_pre"]
                else:
                    db = p5s.tile([128, KI, 128], F16, name="db", tag="db", bufs=3)
                    nc.sync.dma_start(db[:], g["wdn"][:, m, :, :])
                for tb in range(NB):
                    tcols = slice(tb * 512, (tb + 1) * 512)
                    d_ps = p5p.tile([128, 512], F32, name="d_ps", tag="d_ps", bufs=2)
                    for k in range(KI):
                        nc.tensor.matmul(d_ps[:], db[:, k, :], hful[:, k, tcols],
                                         start=(k == 0), stop=(k == KI - 1))
                    ot = p5s.tile([128, 512], F16, name="ot", tag="ot", bufs=2)
                    nc.vector.tensor_copy(ot[:], d_ps[:])
                    nc.sync.dma_start(g[f"rs_in{r}"][mi * 128:(mi + 1) * 128, tcols], ot[:])
            if with_collectives:
                nc.gpsimd.collective_compute(
                    "ReduceScatter", mybir.AluOpType.add, replica_groups=rg,
                    ins=[g[f"rs_in{r}"].opt()], outs=[g[f"rs_out{r}"].opt()],
                )
            else:
                nc.sync.dma_start(g[f"rs_out{r}"][:, :], g[f"rs_in{r}"][0:H // NC // 8, :])
            nc.sync.dma_start(
                g["out_down"][r * 64:(r + 1) * 64, :], g[f"rs_out{r}"][:, :])


def build_program(with_collectives=True, stop_after=99):
    nc = bacc.Bacc("TRN2", target_bir_lowering=False, debug=False, num_devices=NC)

    g = {}
    g["hT"] = nc.dram_tensor("hT", [H, S], F16, kind="ExternalInput")
    g["hT_slice"] = nc.dram_tensor("hT_slice", [128, KH, TPC], F16, kind="ExternalInput")
    g["wqkv"] = nc.dram_tensor("wqkv", [128, KH, (QH + 2) * 128], F16, kind="ExternalInput")
    g["wo"] = nc.dram_tensor("wo", [128, KH, KH, 128], F16, kind="ExternalInput")
    g["wgu"] = nc.dram_tensor("wgu", [128, MB_GU, KH, 256], F16, kind="ExternalInput")
    g["wdn"] = nc.dram_tensor("wdn", [128, KH, KI, 128], F16, kind="ExternalInput")
    g["cosT"] = nc.dram_tensor("cosT", [128, S], F16, kind="ExternalInput")
    g["sinT"] = nc.dram_tensor("sinT", [128, S], F16, kind="ExternalInput")
    g["masks"] = nc.dram_tensor("masks", [128, 128], F16, kind="ExternalInput")

    g["res_out"] = nc.dram_tensor("res_out", [H, TPC], F16, kind="ExternalOutput")
    g["out_down"] = nc.dram_tensor("out_down", [H // NC, S], F16, kind="ExternalOutput")

    rg = [list(range(NC))]

    with tile.TileContext(nc) as tc:
        with (
            tc.tile_pool(name="consts", bufs=1) as consts,
            tc.tile_pool(name="dram", bufs=1, space="DRAM") as dram,
        ):
            for hh in range(QH):
                g[f"a2a_in{hh}"] = dram.tile([NC, 128, TPC], F16, name=f"a2a_in{hh}")
                g[f"a2a_out{hh}"] = dram.tile([NC, 128, TPC], F16, name=f"a2a_out{hh}")
            for kq in range(4):
                g[f"ag2_in_q{kq}"] = dram.tile([128, 8, TPC], F16, name=f"ag2_in_q{kq}")
                g[f"ag2_out_q{kq}"] = dram.tile([NC, 128, 8, TPC], F16,
                                                name=f"ag2_out_q{kq}", addr_space="Shared")
            for r in range(8):
                g[f"rs_in{r}"] = dram.tile([H // 8, S], F16, name=f"rs_in{r}")
                g[f"rs_out{r}"] = dram.tile([H // NC // 8, S], F16, name=f"rs_out{r}")

            ones32 = consts.tile([128, 1], F32, name="ones32")
            nc.gpsimd.memset(ones32[:], 1.0)
            g["ones"] = consts.tile([128, 1], F16, name="ones")
            nc.vector.tensor_copy(g["ones"][:], ones32[:])
            ident32 = consts.tile([128, 128], F32, name="ident32")
            make_identity(nc, ident32[:])
            g["ident"] = consts.tile([128, 128], F16, name="ident")
            nc.vector.tensor_copy(g["ident"][:], ident32[:])
            g["epsb"] = consts.tile([1, 1], F32, name="epsb")
            nc.gpsimd.memset(g["epsb"][:], EPS)

            # h = silu(gate)*up stays SBUF-resident across phases 4 and 5
            with tc.tile_pool(name="mlpkeep", bufs=1) as mlpkeep:
                g["hful"] = mlpkeep.tile([128, KI, S], F16, name="hful")  # 7.3 MB
                g["wdn_pre"] = mlpkeep.tile([128, KI, 128], F16, name="wdn_pre")
                g["gu_pre"] = mlpkeep.tile([128, KH, 256], F16, name="gu_pre")  # 2 MB

                # phase-3 operands that outlive the attention pools
                with tc.tile_pool(name="p3keep", bufs=1) as p3keep:
                    g["hsl"] = p3keep.tile([128, KH, TPC], F16, name="hsl")       # 2 MB
                    g["asl"] = p3keep.tile([128, QH, NC, TPC], F16, name="asl")   # 2 MB
                    g["wo_pre"] = p3keep.tile([128, 3, KH, 128], F16, name="wo_pre")  # 3 MB

                    with tc.tile_pool(name="attn", bufs=1) as attn:
                        g["cos_sb"] = attn.tile([128, S], F16, name="cos_sb")
                        g["sin_sb"] = attn.tile([128, S], F16, name="sin_sb")
                        g["mask_sb"] = attn.tile([128, 128], F16, name="mask_sb")
                        g["qT_sb"] = attn.tile([128, QH, S], F16, name="qT_sb")          # 2 MB
                        g["kT_sb"] = attn.tile([128, S], F16, name="kT_sb")              # 0.5 MB
                        for nb in range(NB):
                            g[f"vT{nb}"] = attn.tile([128, 512], F16, name=f"vT{nb}")
                        g["v_tok"] = attn.tile([128, S // 128, 128], F16, name="v_tok")  # 0.5 MB

                        _phase1_qkv(nc, tc, g)
                        if stop_after >= 2:
                            _phase2_attention(nc, tc, g, with_collectives, rg)

                    if stop_after >= 3:
                        _phase3_oproj(nc, tc, g, with_collectives, rg)

                if stop_after >= 4:
                    _phase4_gate_up(nc, tc, g)

                if stop_after >= 5:
                    _phase5_down(nc, tc, g, with_collectives, rg)

    nc.finalize()
    return nc


_cached_nc = None


def _get_nc():
    global _cached_nc
    if _cached_nc is None:
        _cached_nc = build_program(with_collectives=True)
    return _cached_nc


def _host_prep(positions, hidden_states, w_qkv, w_o, w_gate_up, w_down, ln1_w, ln2_w):
    f32 = np.float32
    f16 = np.float16
    hidden = np.asarray(hidden_states, dtype=f32)[0]          # [S, H]
    hT = np.ascontiguousarray(hidden.T).astype(f16)            # [H, S]
    pos = np.asarray(positions).astype(f32)[0]                 # [S]

    half = HD // 2
    inv_freq = (1.0 / (f32(THETA) ** (np.arange(0, half, dtype=f32) / f32(half)))).astype(f32)
    ang = pos[:, None] * inv_freq[None, :]                     # [S, 64] fp32
    cos_half = np.cos(ang).astype(f32).T                       # [64, S]
    sin_half = np.sin(ang).astype(f32).T
    cosT_np = np.concatenate([cos_half, cos_half], axis=0).astype(f16)  # [128, S]
    sinT_np = np.concatenate([sin_half, sin_half], axis=0).astype(f16)

    w_qkv_f = np.asarray(w_qkv, dtype=f32) * np.asarray(ln1_w, dtype=f32)[:, None]
    w_gu_f = np.asarray(w_gate_up, dtype=f32) * np.asarray(ln2_w, dtype=f32)[:, None]
    # contraction (k') order is head-major: k' = hh*8 + r <-> global head 4r+hh
    kperm = [4 * (k % NC) + (k // NC) for k in range(KH)]
    w_o_f = np.ascontiguousarray(
        np.asarray(w_o, dtype=f32).reshape(KH, 128, KH, 128)
        .transpose(1, 2, 0, 3)[:, :, kperm, :]
    ).astype(f16)
    w_dn_f = np.asarray(w_down, dtype=f32)

    # causal triangle for the leading 128 columns of each diagonal tile
    masks_np = np.ascontiguousarray(
        (np.arange(128)[None, :] >= np.arange(128)[:, None]).astype(f16))  # [128, 128]

    in_maps = []
    for c in range(NC):
        q_cols = w_qkv_f[:, c * QH * HD:(c + 1) * QH * HD]
        k_col = w_qkv_f[:, NQ * HD + c * HD: NQ * HD + (c + 1) * HD]
        v_col = w_qkv_f[:, (NQ + NKV) * HD + c * HD: (NQ + NKV) * HD + (c + 1) * HD]
        wqkv_c = np.concatenate([q_cols, k_col, v_col], axis=1)
        wqkv_c = np.ascontiguousarray(
            wqkv_c.reshape(KH, 128, (QH + 2) * 128).transpose(1, 0, 2)).astype(f16)
        # per-m interleave: [128, m, k, gate128|up128]
        wg_c = w_gu_f[:, c * IPC:(c + 1) * IPC].reshape(KH, 128, MB_GU, 128)
        wu_c = w_gu_f[:, I + c * IPC: I + (c + 1) * IPC].reshape(KH, 128, MB_GU, 128)
        wgu_c = np.ascontiguousarray(
            np.concatenate([wg_c[..., None, :], wu_c[..., None, :]], axis=3)
            .reshape(KH, 128, MB_GU, 256).transpose(1, 2, 0, 3)).astype(f16)
        wdn_c = np.ascontiguousarray(
            w_dn_f[c * IPC:(c + 1) * IPC, :].reshape(KI, 128, KH, 128)
            .transpose(1, 2, 0, 3)).astype(f16)
        hT_slice_c = np.ascontiguousarray(
            hT[:, c * TPC:(c + 1) * TPC].reshape(KH, 128, TPC).transpose(1, 0, 2))
        in_maps.append({
            "hT": hT,
            "hT_slice": hT_slice_c,
            "wqkv": wqkv_c,
            "wo": w_o_f,
            "wgu": wgu_c,
            "wdn": wdn_c,
            "cosT": cosT_np,
            "sinT": sinT_np,
            "masks": masks_np,
        })
    return in_maps


def kernel(**inputs):
    in_maps = _host_prep(**inputs)
    nc = _get_nc()
    res = run_bass_kernel_spmd(nc, in_maps, core_ids=list(range(NC)))
    results = res.results

    outT = np.empty((H, S), np.float32)
    for c in range(NC):
        od = results[c]["out_down"].astype(np.float32)  # [512, S]
        for r in range(8):
            outT[512 * r + 64 * c: 512 * r + 64 * (c + 1)] = od[64 * r:64 * (r + 1)]
    resT = np.concatenate(
        [results[c]["res_out"].astype(np.float32) for c in range(NC)], axis=1)  # [H, S]
    out = np.ascontiguousarray(outT.T).reshape(1, S, H).astype(np.float32)
    residual = np.ascontiguousarray(resT.T).reshape(1, S, H).astype(np.float32)
    return out, residual


# revision 67
# speedup vs baseline: 1.0701x; 1.0029x over previous
"""Bamba attention decoder layer on 8 Trainium2 NeuronCores.

Sharding: tensor-parallel attention (4 q heads + 1 kv head per core),
AllToAll of attention context (delivers each core its token slice at a static
address), token-sliced o_proj + fused add/rmsnorm, AllGather of normed
activations, I-sharded SwiGLU MLP (1792 cols/core), ReduceScatter of
down-proj partials.

Layout: feature-major activations ([features->partitions, tokens->free]) so
every linear layer uses its natural-layout weight block as the stationary
matmul operand. All activations/weights are fp16 (psum accumulation stays
fp32): halves DMA traffic and doubles DVE element throughput at unchanged PE
rate. The 1/rms factor of rmsnorm1 is applied after the QKV matmul (per-token
column scaling commutes through the contraction); ln weights are folded into
the weight matrices on the host.

Schedule notes (vs the original fp32 version):
- qkv weight chunks are interleaved with the first token block's hb loads so
  the PE starts ~5us in instead of waiting for the full 12.6MB.
- all 6 qkv psum banks are evacuated by immediate scalar-engine copies; the
  rmsnorm/rope chain runs from SBUF off the PE critical path, and the V
  transposes are deferred to a mini-phase after the last block.
- attention processes score tiles in pairs ([128,1024] psum tiles) to halve
  exp/mask/accumulate instruction count; probs in fp16.
- o_proj weights (fp16) stream under the matmul; the first 4 m-tiles and the
  residual slice prefetch during attention.
- the MLP first matmul is single-pass (all 2048 tokens resident in fp16), so
  gate/up weights load once; h round-trips DRAM in fp16 with the phase-5
  reload chunked per k-tile so it overlaps phase 4.
"""

import numpy as np

import concourse.bacc as bacc
import concourse.mybir as mybir
import concourse.tile as tile
from concourse.bass_utils import run_bass_kernel_spmd
from concourse.masks import make_identity

NC = 8
S = 2048
H = 4096
HD = 128
NQ = 32
NKV = 8
I = 14336
QH = NQ // NC        # q heads per core = 4
IPC = I // NC        # intermediate cols per core = 1792
TPC = S // NC        # tokens per core = 256
EPS = 1e-5
THETA = 10000.0
SCALE = HD ** -0.5

F32 = mybir.dt.float32
F16 = mybir.dt.float16

KH = H // 128        # 32 k-tiles over H
NB = S // 512        # 4 token blocks of 512
MB_GU = IPC // 128   # 14 m tiles for gate (and for up)
KI = IPC // 128      # 14 k tiles over I per core

AF = mybir.ActivationFunctionType


def _phase1_qkv(nc, tc, g):
    """QKV matmul + rmsnorm1 stats + rope. Fills qT_sb/kT_sb/vT_sb."""
    with (
        tc.tile_pool(name="p1sbuf", bufs=2) as p1s,
        tc.tile_pool(name="p1psum", bufs=1, space="PSUM") as p1p,
    ):
        tp1 = p1p.tile([128, 8, 128], F16, name="tp1", tag="tp1")
        for nb in range(NB):
            if nb >= 2:
                # V transposes for block nb-2 (data long ready, so the PE
                # passes through them without stalling)
                for t in range(4 * (nb - 2), 4 * (nb - 1)):
                    _vtok_one(nc, tp1, g, t)
            ncols = slice(nb * 512, (nb + 1) * 512)
            st_ps = p1p.tile([1, 512], F32, name="st_ps", tag="st_ps")
            # one psum tile per output block so dependency tracking stays
            # per-bank: the next token block's matmul for slot m waits only
            # on slot m's evacuation copy
            mm_ps = [p1p.tile([128, 512], F32, name=f"mm_ps{m}", tag=f"mm_ps{m}")
                     for m in range(QH + 2)]
            for k in range(KH):
                # weights stream alongside the activations (re-read per
                # block; DMA has slack and this keeps SBUF residency low)
                wqk = p1s.tile([128, (QH + 2) * 128], F16, name="wqk",
                               tag="wqk", bufs=6)
                nc.sync.dma_start(wqk[:], g["wqkv"][:, k, :])
                hb = p1s.tile([128, 512], F16, name="hb", tag="hb", bufs=5)
                nc.sync.dma_start(hb[:], g["hT"][k * 128:(k + 1) * 128, ncols])
                sq = p1s.tile([128, 512], F16, name="sq", tag="sq", bufs=3)
                nc.scalar.activation(sq[:], hb[:], AF.Square)
                nc.tensor.matmul(st_ps[:], g["ones"][:], sq[:],
                                 start=(k == 0), stop=(k == KH - 1))
                for m in range(QH + 2):
                    nc.tensor.matmul(
                        mm_ps[m][:], wqk[:, m * 128:(m + 1) * 128], hb[:],
                        start=(k == 0), stop=(k == KH - 1),
                    )
                if k == 20:
                    # rope tables for this block, needed right after the
                    # k-loop (spread across blocks to keep DMA bursts small)
                    nc.sync.dma_start(g["cos_sb"][:, ncols], g["cosT"][:, ncols])
                    nc.sync.dma_start(g["sin_sb"][:, ncols], g["sinT"][:, ncols])
                    if nb == 0:
                        nc.sync.dma_start(g["mask_sb"][:], g["masks"][:, :])
            # evacuate all 6 psum banks immediately (scalar engine) so the
            # next block's matmuls only wait on these copies, not on the
            # rmsnorm/rope chain below
            qkc = p1s.tile([128, QH + 2, 512], F16, name="qkc", tag="qkc", bufs=1)
            for m in range(QH + 2):
                nc.vector.tensor_copy(qkc[:, m, :], mm_ps[m][:])
            std_row = p1s.tile([1, 512], F32, name="std_row", tag="std_row")
            nc.scalar.activation(std_row[:], st_ps[:], AF.Sqrt,
                                 bias=g["epsb"][:], scale=1.0 / H)
            rstd_row = p1s.tile([1, 512], F32, name="rstd_row", tag="rstd_row")
            nc.vector.reciprocal(rstd_row[:], std_row[:])
            rstd16 = p1s.tile([1, 512], F16, name="rstd16", tag="rstd16")
            nc.vector.tensor_copy(rstd16[:], rstd_row[:])
            rb = p1s.tile([128, 512], F16, name="rb", tag="rb", bufs=1)
            nc.gpsimd.partition_broadcast(rb[:], rstd16[:])
            if nb == NB - 1:
                # dummy op to pre-load the Exp activation table during the
                # rope tail, so phase 2's first exp pays no table swap
                nc.scalar.activation(g["dummy"][:], g["epsb"][:], AF.Exp)
            cos_s = p1s.tile([128, 512], F16, name="cos_s", tag="cos_s", bufs=1)
            nc.vector.tensor_mul(cos_s[:], g["cos_sb"][:, ncols], rb[:])
            sin_s = p1s.tile([128, 512], F16, name="sin_s", tag="sin_s", bufs=1)
            nc.vector.tensor_mul(sin_s[:], g["sin_sb"][:, ncols], rb[:])
            for m in range(QH + 1):
                if m < QH:
                    d0 = g["qT_sb"][0:64, m, ncols]
                    d1 = g["qT_sb"][64:128, m, ncols]
                else:
                    d0 = g["kT_sb"][0:64, ncols]
                    d1 = g["kT_sb"][64:128, ncols]
                t0 = p1s.tile([64, 512], F16, name="t0", tag="t0", bufs=1)
                nc.vector.tensor_mul(t0[:], qkc[0:64, m, :], cos_s[0:64, :])
                t1 = p1s.tile([64, 512], F16, name="t1", tag="t1", bufs=1)
                nc.vector.tensor_mul(t1[:], qkc[64:128, m, :], sin_s[64:128, :])
                nc.vector.tensor_sub(d0, t0[:], t1[:])
                t2 = p1s.tile([64, 512], F16, name="t2", tag="t0", bufs=1)
                nc.vector.tensor_mul(t2[:], qkc[64:128, m, :], cos_s[64:128, :])
                t3 = p1s.tile([64, 512], F16, name="t3", tag="t1", bufs=1)
                nc.vector.tensor_mul(t3[:], qkc[0:64, m, :], sin_s[0:64, :])
                nc.vector.tensor_add(d1, t2[:], t3[:])
            nc.vector.tensor_mul(g[f"vT{nb}"][:, :], qkc[:, QH + 1, :], rb[:])

        # V transposes for blocks 0-1 (later blocks are deferred into phase 2
        # so their rmsnorm chains don't stall the PE between phase 1 and 2)
        tp1 = p1p.tile([128, 8, 128], F16, name="tp1", tag="tp1")
        for t in range(8):
            _vtok_one(nc, tp1, g, t)


def _vtok_one(nc, tp, g, t):
    """Transpose one 128-token V tile to token-major for the PV matmuls."""
    nb, j = t // 4, t % 4
    nc.tensor.transpose(tp[:, t % 8, :], g[f"vT{nb}"][:, j * 128:(j + 1) * 128],
                        g["ident"][:])
    nc.vector.tensor_copy(g["v_tok"][:, t, :], tp[:, t % 8, :])


def _phase2_attention(nc, tc, g, with_collectives, rg):
    with (
        tc.tile_pool(name="p2sbuf", bufs=2) as p2s,
        tc.tile_pool(name="p2psum", bufs=1, space="PSUM") as p2p,
    ):
        # prefetch phase-3/4/5 operands with no dependency on attention
        # (phase 2 has plenty of DMA slack)
        nc.sync.dma_start(g["hsl"][:], g["hT_slice"][:, :, :])
        for m in range(3):
            nc.sync.dma_start(g["wo_pre"][:, m, :, :], g["wo"][:, m, :, :])
        nc.sync.dma_start(g["gu_pre"][:], g["wgu"][:, 0, :, :])
        nc.sync.dma_start(g["wdn_pre"][:], g["wdn"][:, 0, :, :])

        for hh in range(QH):
            anob = p2s.tile([128, NB, 512], F16, name="anob", tag="anob", bufs=2)
            for qb in range(NB):
                if hh == 0 and qb == 1:
                    # blocks 2-3's V transposes: emitted after the first score
                    # block so the PE never waits on their rmsnorm chains
                    tp2 = p2p.tile([128, 8, 128], F16, name="tp2", tag="tp2")
                    for t in range(8, 16):
                        _vtok_one(nc, tp2, g, t)
                qcols = slice(qb * 512, (qb + 1) * 512)
                att_ps = p2p.tile([128, 512], F32, name="att_ps", tag="att_ps", bufs=2)
                acc = p2s.tile([128, 2, 512], F16, name="acc", tag="acc", bufs=2)
                # full (unmasked) score tiles, processed in pairs
                for pp in range(2 * qb):
                    kt0, kt1 = 2 * pp, 2 * pp + 1
                    s_ps = p2p.tile([128, 2, 512], F32, name="s_ps", tag="s_ps", bufs=2)
                    nc.tensor.matmul(
                        s_ps[:, 0, :], g["kT_sb"][:, kt0 * 128:(kt0 + 1) * 128],
                        g["qT_sb"][:, hh, qcols], start=True, stop=True,
                    )
                    nc.tensor.matmul(
                        s_ps[:, 1, :], g["kT_sb"][:, kt1 * 128:(kt1 + 1) * 128],
                        g["qT_sb"][:, hh, qcols], start=True, stop=True,
                    )
                    e = p2s.tile([128, 2, 512], F16, name="e", tag="e", bufs=4)
                    nc.scalar.activation(e[:], s_ps[:], AF.Exp, scale=SCALE)
                    if pp == 0:
                        nc.vector.tensor_copy(acc[:], e[:])
                    else:
                        nc.vector.tensor_add(acc[:], acc[:], e[:])
                    nc.tensor.matmul(att_ps[:], g["v_tok"][:, kt0, :], e[:, 0, :],
                                     start=(pp == 0), stop=False)
                    nc.tensor.matmul(att_ps[:], g["v_tok"][:, kt1, :], e[:, 1, :],
                                     start=False, stop=False)
                # the 4 diagonal tiles: causality restricts tile j to local
                # q >= 128j, so matmul/exp/accumulate only the valid width and
                # apply a 128x128 triangle mask to the leading sub-block
                for pj in range(2):
                    s_ps = p2p.tile([128, 2, 512], F32, name="s_ps", tag="s_ps", bufs=2)
                    e = p2s.tile([128, 2, 512], F16, name="e", tag="e", bufs=4)
                    for i in range(2):
                        j = 2 * pj + i
                        kt = 4 * qb + j
                        qoff, w = 128 * j, 512 - 128 * j
                        nc.tensor.matmul(
                            s_ps[:, i, 0:w], g["kT_sb"][:, kt * 128:(kt + 1) * 128],
                            g["qT_sb"][:, hh, qb * 512 + qoff:(qb + 1) * 512],
                            start=True, stop=True,
                        )
                    if pj == 0:
                        # j=0 is full width; j=1 region [384:512) holds stale
                        # psum, exp'd but never read
                        nc.scalar.activation(e[:], s_ps[:], AF.Exp, scale=SCALE)
                    else:
                        nc.scalar.activation(e[:, 0, 0:256], s_ps[:, 0, 0:256],
                                             AF.Exp, scale=SCALE)
                        nc.scalar.activation(e[:, 1, 0:128], s_ps[:, 1, 0:128],
                                             AF.Exp, scale=SCALE)
                    for i in range(2):
                        j = 2 * pj + i
                        kt = 4 * qb + j
                        qoff, w = 128 * j, 512 - 128 * j
                        nc.vector.tensor_mul(e[:, i, 0:128], e[:, i, 0:128],
                                             g["mask_sb"][:, :])
                        if qb == 0 and j == 0:
                            nc.vector.tensor_copy(acc[:, 0, :], e[:, 0, :])
                        elif qb == 0 and j == 1:
                            nc.gpsimd.memset(acc[:, 1, 0:128], 0.0)
                            nc.vector.tensor_copy(acc[:, 1, 128:512], e[:, 1, 0:384])
                        else:
                            nc.vector.tensor_add(acc[:, i, qoff:512],
                                                 acc[:, i, qoff:512], e[:, i, 0:w])
                        nc.tensor.matmul(
                            att_ps[:, qoff:512], g["v_tok"][:, kt, :], e[:, i, 0:w],
                            start=(qb == 0 and j == 0), stop=(j == 3),
                        )
                sums_ps = p2p.tile([1, 512], F32, name="sums_ps", tag="sums_ps", bufs=1)
                nc.tensor.matmul(sums_ps[:], g["ones"][:], acc[:, 0, :],
                                 start=True, stop=False)
                nc.tensor.matmul(sums_ps[:], g["ones"][:], acc[:, 1, :],
                                 start=False, stop=True)
                recip = p2s.tile([1, 512], F32, name="recip", tag="recip")
                nc.vector.reciprocal(recip[:], sums_ps[:])
                recip16 = p2s.tile([1, 512], F16, name="recip16", tag="recip16")
                nc.vector.tensor_copy(recip16[:], recip[:])
                rb2 = p2s.tile([128, 512], F16, name="rb2", tag="rb2", bufs=2)
                nc.gpsimd.partition_broadcast(rb2[:], recip16[:])
                nc.vector.tensor_mul(anob[:, qb, :], att_ps[:], rb2[:])
            # one batched DMA scatters this head's context into the A2A
            # input layout ([core, 128, 256]); per-partition runs stay 512B
            nc.sync.dma_start(
                g[f"a2a_in{hh}"][:, :, :].transpose([1, 0, 2]),
                anob[:, :, :],
            )
            # ship this head's context while the next head computes
            if with_collectives:
                nc.gpsimd.collective_compute(
                    "AllToAll", mybir.AluOpType.bypass, replica_groups=rg,
                    ins=[g[f"a2a_in{hh}"].opt()], outs=[g[f"a2a_out{hh}"].opt()],
                )
                # pull this head's context into the o_proj operand layout
                nc.sync.dma_start(
                    g["asl"][:, hh, :, :],
                    g[f"a2a_out{hh}"][:, :, :].transpose([1, 0, 2]),
                )
            else:
                # local stand-in: same byte count, sourced from SBUF so the
                # o_proj operand isn't chained behind the a2a_in write
                nc.sync.dma_start(g["asl"][:, hh, :, :], anob[:, :, :])


def _phase3_oproj(nc, tc, g, with_collectives, rg):
    with (
        tc.tile_pool(name="p3sbuf", bufs=2) as p3s,
        tc.tile_pool(name="p3psum", bufs=1, space="PSUM") as p3p,
    ):
        res2 = p3s.tile([128, KH, TPC], F16, name="res2", tag="res2", bufs=1)
        st2_ps = p3p.tile([1, TPC], F32, name="st2_ps", tag="st2_ps")
        asl = g["asl"]
        # dummy op: pre-load the Sqrt table while the scalar engine is idle,
        # so the rmsnorm2 tail pays no table swap
        nc.scalar.activation(g["dummy"][:], g["epsb"][:], AF.Sqrt)
        for m in range(KH):
            if m < 3:
                wsrc = g["wo_pre"][:, m]
            else:
                wob = p3s.tile([128, KH, 128], F16, name="wob", tag="wob", bufs=3)
                nc.sync.dma_start(wob[:], g["wo"][:, m, :, :])
                wsrc = wob
            o_ps = p3p.tile([128, TPC], F32, name="o_ps", tag="o_ps", bufs=2)
            for k in range(KH):
                nc.tensor.matmul(o_ps[:], wsrc[:, k, :], asl[:, k // 8, k % 8, :],
                                 start=(k == 0), stop=(k == KH - 1))
            nc.vector.tensor_add(res2[:, m, :], o_ps[:], g["hsl"][:, m, :])
            nc.sync.dma_start(g["res_out"][m * 128:(m + 1) * 128, :], res2[:, m, :])
            sq2 = p3s.tile([128, TPC], F16, name="sq2", tag="sq2", bufs=2)
            nc.vector.tensor_mul(sq2[:], res2[:, m, :], res2[:, m, :])
            nc.tensor.matmul(st2_ps[:], g["ones"][:], sq2[:],
                             start=(m == 0), stop=(m == KH - 1))
        std2 = p3s.tile([1, TPC], F32, name="std2", tag="std2")
        nc.scalar.activation(std2[:], st2_ps[:], AF.Sqrt, bias=g["epsb"][:], scale=1.0 / H)
        rstd2 = p3s.tile([1, TPC], F32, name="rstd2", tag="rstd2")
        nc.vector.reciprocal(rstd2[:], std2[:])
        rstd2_16 = p3s.tile([1, TPC], F16, name="rstd2_16", tag="rstd2_16")
        nc.vector.tensor_copy(rstd2_16[:], rstd2[:])
        rb3 = p3s.tile([128, TPC], F16, name="rb3", tag="rb3")
        nc.gpsimd.partition_broadcast(rb3[:], rstd2_16[:])
        x2bs = []
        for kq in range(4):
            # batch 8 feature-tiles of normed activations into one DMA, then
            # ship the quarter so phase 4's operands stream in incrementally
            x2b = p3s.tile([128, 8, TPC], F16, name="x2b", tag="x2b", bufs=4)
            x2bs.append(x2b)
            for mi in range(8):
                nc.vector.tensor_mul(x2b[:, mi, :], res2[:, kq * 8 + mi, :], rb3[:])
            if with_collectives:
                nc.sync.dma_start(g[f"ag2_in_q{kq}"][:, :, :], x2b[:, :, :])
                nc.gpsimd.collective_compute(
                    "AllGather", mybir.AluOpType.bypass, replica_groups=rg,
                    ins=[g[f"ag2_in_q{kq}"].opt()],
                    outs=[g[f"ag2_out_q{kq}"].opt()],
                )
        g["x2bs"] = x2bs


def _phase4_gate_up(nc, tc, g, with_collectives):
    """SwiGLU first half, one 512-token quarter at a time.

    Each quarter's activations (4.2MB fp16) double-buffer against the previous
    quarter's compute; gate/up weights re-stream per quarter (DMA has slack).
    h = silu(gate)*up is written straight to the SBUF-resident hful tile, so
    phase 5 starts with everything already on-chip.
    """
    with (
        tc.tile_pool(name="p4x", bufs=1) as p4x,
        tc.tile_pool(name="p4sbuf", bufs=2) as p4s,
        tc.tile_pool(name="p4psum", bufs=1, space="PSUM") as p4p,
    ):
        for tb in range(NB):
            tcols = slice(tb * 512, (tb + 1) * 512)
            x2q = p4x.tile([128, KH, 512], F16, name="x2q", tag="x2q", bufs=2)
            if with_collectives:
                for kq in range(4):
                    # one DMA per feature-quarter (both source cores adjacent
                    # in the gather buffer) so the k-loop starts on quarter 0
                    nc.sync.dma_start(
                        x2q[:, kq * 8:(kq + 1) * 8, :],
                        g[f"ag2_out_q{kq}"][2 * tb:2 * tb + 2, :, :, :]
                        .transpose([1, 2, 0, 3]),
                    )
            else:
                # local stand-in for the gather with the same per-core DMA
                # byte count as the real path: this core's slice straight
                # from SBUF, the peer slice from the (unwritten) gather
                # buffer; the collective-input writes are deferred behind the
                # first quarter's operands
                for kq in range(4):
                    if tb == 0:
                        nc.sync.dma_start(x2q[:, kq * 8:(kq + 1) * 8, 0:256],
                                          g["x2bs"][kq][:, :, :])
                        nc.sync.dma_start(x2q[:, kq * 8:(kq + 1) * 8, 256:512],
                                          g[f"ag2_out_q{kq}"][1, :, :, :])
                    else:
                        nc.sync.dma_start(
                            x2q[:, kq * 8:(kq + 1) * 8, :],
                            g[f"ag2_out_q{kq}"][2 * tb:2 * tb + 2, :, :, :]
                            .transpose([1, 2, 0, 3]),
                        )
                if tb == 0:
                    for kq in range(4):
                        nc.sync.dma_start(g[f"ag2_in_q{kq}"][:, :, :],
                                          g["x2bs"][kq][:, :, :])
                        nc.sync.dma_start(g[f"ag2_out_q{kq}"][0, :, :, :],
                                          g["x2bs"][kq][:, :, :])
            for m in range(MB_GU):
                if m == 0:
                    gu = g["gu_pre"]
                else:
                    gu = p4s.tile([128, KH, 256], F16, name="gu", tag="gu", bufs=2)
                    nc.sync.dma_start(gu[:], g["wgu"][:, m, :, :])
                g_ps = p4p.tile([128, 512], F32, name="g_ps", tag="g_ps", bufs=2)
                for k in range(KH):
                    nc.tensor.matmul(g_ps[:], gu[:, k, 0:128], x2q[:, k, :],
                                     start=(k == 0), stop=(k == KH - 1))
                u_ps = p4p.tile([128, 512], F32, name="u_ps", tag="u_ps", bufs=2)
                for k in range(KH):
                    nc.tensor.matmul(u_ps[:], gu[:, k, 128:256], x2q[:, k, :],
                                     start=(k == 0), stop=(k == KH - 1))
                sg = p4s.tile([128, 512], F16, name="sg", tag="sg", bufs=2)
                nc.scalar.activation(sg[:], g_ps[:], AF.Silu)
                nc.vector.tensor_mul(g["hful"][:, m, tcols], sg[:], u_ps[:])


def _phase5_down(nc, tc, g, with_collectives, rg):
    with (
        tc.tile_pool(name="p5sbuf", bufs=2) as p5s,
        tc.tile_pool(name="p5psum", bufs=1, space="PSUM") as p5p,
    ):
        hful = g["hful"]
        for r in range(8):
            for mi in range(KH // 8):
                m = r * (KH // 8) + mi
                if m == 0:
                    db = g["wdn_pre"]
                else:
                    db = p5s.tile([128, KI, 128], F16, name="db", tag="db", bufs=3)
                    nc.sync.dma_start(db[:], g["wdn"][:, m, :, :])
                for tb in range(NB):
                    tcols = slice(tb * 512, (tb + 1) * 512)
                    d_ps = p5p.tile([128, 512], F32, name="d_ps", tag="d_ps", bufs=2)
                    for k in range(KI):
                        nc.tensor.matmul(d_ps[:], db[:, k, :], hful[:, k, tcols],
                                         start=(k == 0), stop=(k == KI - 1))
                    ot = p5s.tile([128, 512], F16, name="ot", tag="ot", bufs=2)
                    nc.vector.tensor_copy(ot[:], d_ps[:])
                    nc.sync.dma_start(g[f"rs_in{r}"][mi * 128:(mi + 1) * 128, tcols], ot[:])
            if with_collectives:
                nc.gpsimd.collective_compute(
                    "ReduceScatter", mybir.AluOpType.add, replica_groups=rg,
                    ins=[g[f"rs_in{r}"].opt()], outs=[g[f"rs_out{r}"].opt()],
                )
            else:
                nc.sync.dma_start(g[f"rs_out{r}"][:, :], g[f"rs_in{r}"][0:H // NC // 8, :])
            nc.sync.dma_start(
                g["out_down"][r * 64:(r + 1) * 64, :], g[f"rs_out{r}"][:, :])


def build_program(with_collectives=True, stop_after=99):
    nc = bacc.Bacc("TRN2", target_bir_lowering=False, debug=False, num_devices=NC)

    g = {}
    g["hT"] = nc.dram_tensor("hT", [H, S], F16, kind="ExternalInput")
    g["hT_slice"] = nc.dram_tensor("hT_slice", [128, KH, TPC], F16, kind="ExternalInput")
    g["wqkv"] = nc.dram_tensor("wqkv", [128, KH, (QH + 2) * 128], F16, kind="ExternalInput")
    g["wo"] = nc.dram_tensor("wo", [128, KH, KH, 128], F16, kind="ExternalInput")
    g["wgu"] = nc.dram_tensor("wgu", [128, MB_GU, KH, 256], F16, kind="ExternalInput")
    g["wdn"] = nc.dram_tensor("wdn", [128, KH, KI, 128], F16, kind="ExternalInput")
    g["cosT"] = nc.dram_tensor("cosT", [128, S], F16, kind="ExternalInput")
    g["sinT"] = nc.dram_tensor("sinT", [128, S], F16, kind="ExternalInput")
    g["masks"] = nc.dram_tensor("masks", [128, 128], F16, kind="ExternalInput")

    g["res_out"] = nc.dram_tensor("res_out", [H, TPC], F16, kind="ExternalOutput")
    g["out_down"] = nc.dram_tensor("out_down", [H // NC, S], F16, kind="ExternalOutput")

    rg = [list(range(NC))]

    with tile.TileContext(nc) as tc:
        with (
            tc.tile_pool(name="consts", bufs=1) as consts,
            tc.tile_pool(name="dram", bufs=1, space="DRAM") as dram,
        ):
            for hh in range(QH):
                g[f"a2a_in{hh}"] = dram.tile([NC, 128, TPC], F16, name=f"a2a_in{hh}")
                g[f"a2a_out{hh}"] = dram.tile([NC, 128, TPC], F16, name=f"a2a_out{hh}")
            for kq in range(4):
                g[f"ag2_in_q{kq}"] = dram.tile([128, 8, TPC], F16, name=f"ag2_in_q{kq}")
                g[f"ag2_out_q{kq}"] = dram.tile([NC, 128, 8, TPC], F16,
                                                name=f"ag2_out_q{kq}", addr_space="Shared")
            for r in range(8):
                g[f"rs_in{r}"] = dram.tile([H // 8, S], F16, name=f"rs_in{r}")
                g[f"rs_out{r}"] = dram.tile([H // NC // 8, S], F16, name=f"rs_out{r}")

            ones32 = consts.tile([128, 1], F32, name="ones32")
            nc.gpsimd.memset(ones32[:], 1.0)
            g["ones"] = consts.tile([128, 1], F16, name="ones")
            nc.vector.tensor_copy(g["ones"][:], ones32[:])
            ident32 = consts.tile([128, 128], F32, name="ident32")
            make_identity(nc, ident32[:])
            g["ident"] = consts.tile([128, 128], F16, name="ident")
            nc.vector.tensor_copy(g["ident"][:], ident32[:])
            g["epsb"] = consts.tile([1, 1], F32, name="epsb")
            nc.gpsimd.memset(g["epsb"][:], EPS)
            g["dummy"] = consts.tile([1, 1], F32, name="dummy")

            # h = silu(gate)*up stays SBUF-resident across phases 4 and 5
            with tc.tile_pool(name="mlpkeep", bufs=1) as mlpkeep:
                g["hful"] = mlpkeep.tile([128, KI, S], F16, name="hful")  # 7.3 MB
                g["wdn_pre"] = mlpkeep.tile([128, KI, 128], F16, name="wdn_pre")
                g["gu_pre"] = mlpkeep.tile([128, KH, 256], F16, name="gu_pre")  # 2 MB

                # phase-3 operands that outlive the attention pools
                with tc.tile_pool(name="p3keep", bufs=1) as p3keep:
                    g["hsl"] = p3keep.tile([128, KH, TPC], F16, name="hsl")       # 2 MB
                    g["asl"] = p3keep.tile([128, QH, NC, TPC], F16, name="asl")   # 2 MB
                    g["wo_pre"] = p3keep.tile([128, 3, KH, 128], F16, name="wo_pre")  # 3 MB

                    with tc.tile_pool(name="attn", bufs=1) as attn:
                        g["cos_sb"] = attn.tile([128, S], F16, name="cos_sb")
                        g["sin_sb"] = attn.tile([128, S], F16, name="sin_sb")
                        g["mask_sb"] = attn.tile([128, 128], F16, name="mask_sb")
                        g["qT_sb"] = attn.tile([128, QH, S], F16, name="qT_sb")          # 2 MB
                        g["kT_sb"] = attn.tile([128, S], F16, name="kT_sb")              # 0.5 MB
                        for nb in range(NB):
                            g[f"vT{nb}"] = attn.tile([128, 512], F16, name=f"vT{nb}")
                        g["v_tok"] = attn.tile([128, S // 128, 128], F16, name="v_tok")  # 0.5 MB

                        _phase1_qkv(nc, tc, g)
                        if stop_after >= 2:
                            _phase2_attention(nc, tc, g, with_collectives, rg)

                    if stop_after >= 3:
                        _phase3_oproj(nc, tc, g, with_collectives, rg)

                if stop_after >= 4:
                    _phase4_gate_up(nc, tc, g, with_collectives)

                if stop_after >= 5:
                    _phase5_down(nc, tc, g, with_collectives, rg)

    nc.finalize()
    return nc


_cached_nc = None


def _get_nc():
    global _cached_nc
    if _cached_nc is None:
        _cached_nc = build_program(with_collectives=True)
    return _cached_nc


def _host_prep(positions, hidden_states, w_qkv, w_o, w_gate_up, w_down, ln1_w, ln2_w):
    f32 = np.float32
    f16 = np.float16
    hidden = np.asarray(hidden_states, dtype=f32)[0]          # [S, H]
    hT = np.ascontiguousarray(hidden.T).astype(f16)            # [H, S]
    pos = np.asarray(positions).astype(f32)[0]                 # [S]

    half = HD // 2
    inv_freq = (1.0 / (f32(THETA) ** (np.arange(0, half, dtype=f32) / f32(half)))).astype(f32)
    ang = pos[:, None] * inv_freq[None, :]                     # [S, 64] fp32
    cos_half = np.cos(ang).astype(f32).T                       # [64, S]
    sin_half = np.sin(ang).astype(f32).T
    cosT_np = np.concatenate([cos_half, cos_half], axis=0).astype(f16)  # [128, S]
    sinT_np = np.concatenate([sin_half, sin_half], axis=0).astype(f16)

    w_qkv_f = np.asarray(w_qkv, dtype=f32) * np.asarray(ln1_w, dtype=f32)[:, None]
    w_gu_f = np.asarray(w_gate_up, dtype=f32) * np.asarray(ln2_w, dtype=f32)[:, None]
    # contraction (k') order is head-major: k' = hh*8 + r <-> global head 4r+hh
    kperm = [4 * (k % NC) + (k // NC) for k in range(KH)]
    w_o_f = np.ascontiguousarray(
        np.asarray(w_o, dtype=f32).reshape(KH, 128, KH, 128)
        .transpose(1, 2, 0, 3)[:, :, kperm, :]
    ).astype(f16)
    w_dn_f = np.asarray(w_down, dtype=f32)

    # causal triangle for the leading 128 columns of each diagonal tile
    masks_np = np.ascontiguousarray(
        (np.arange(128)[None, :] >= np.arange(128)[:, None]).astype(f16))  # [128, 128]

    in_maps = []
    for c in range(NC):
        q_cols = w_qkv_f[:, c * QH * HD:(c + 1) * QH * HD]
        k_col = w_qkv_f[:, NQ * HD + c * HD: NQ * HD + (c + 1) * HD]
        v_col = w_qkv_f[:, (NQ + NKV) * HD + c * HD: (NQ + NKV) * HD + (c + 1) * HD]
        wqkv_c = np.concatenate([q_cols, k_col, v_col], axis=1)
        wqkv_c = np.ascontiguousarray(
            wqkv_c.reshape(KH, 128, (QH + 2) * 128).transpose(1, 0, 2)).astype(f16)
        # per-m interleave: [128, m, k, gate128|up128]
        wg_c = w_gu_f[:, c * IPC:(c + 1) * IPC].reshape(KH, 128, MB_GU, 128)
        wu_c = w_gu_f[:, I + c * IPC: I + (c + 1) * IPC].reshape(KH, 128, MB_GU, 128)
        wgu_c = np.ascontiguousarray(
            np.concatenate([wg_c[..., None, :], wu_c[..., None, :]], axis=3)
            .reshape(KH, 128, MB_GU, 256).transpose(1, 2, 0, 3)).astype(f16)
        wdn_c = np.ascontiguousarray(
            w_dn_f[c * IPC:(c + 1) * IPC, :].reshape(KI, 128, KH, 128)
            .transpose(1, 2, 0, 3)).astype(f16)
        hT_slice_c = np.ascontiguousarray(
            hT[:, c * TPC:(c + 1) * TPC].reshape(KH, 128, TPC).transpose(1, 0, 2))
        in_maps.append({
            "hT": hT,
            "hT_slice": hT_slice_c,
            "wqkv": wqkv_c,
            "wo": w_o_f,
            "wgu": wgu_c,
            "wdn": wdn_c,
            "cosT": cosT_np,
            "sinT": sinT_np,
            "masks": masks_np,
        })
    return in_maps


def kernel(**inputs):
    in_maps = _host_prep(**inputs)
    nc = _get_nc()
    res = run_bass_kernel_spmd(nc, in_maps, core_ids=list(range(NC)))
    results = res.results

    outT = np.empty((H, S), np.float32)
    for c in range(NC):
        od = results[c]["out_down"].astype(np.float32)  # [512, S]
        for r in range(8):
            outT[512 * r + 64 * c: 512 * r + 64 * (c + 1)] = od[64 * r:64 * (r + 1)]
    resT = np.concatenate(
        [results[c]["res_out"].astype(np.float32) for c in range(NC)], axis=1)  # [H, S]
    out = np.ascontiguousarray(outT.T).reshape(1, S, H).astype(np.float32)
    residual = np.ascontiguousarray(resT.T).reshape(1, S, H).astype(np.float32)
    return out, residual


# revision 74
# speedup vs baseline: 1.0789x; 1.0083x over previous
"""Bamba attention decoder layer on 8 Trainium2 NeuronCores.

Sharding: tensor-parallel attention (4 q heads + 1 kv head per core),
AllToAll of attention context (delivers each core its token slice at a static
address), token-sliced o_proj + fused add/rmsnorm, AllGather of normed
activations, I-sharded SwiGLU MLP (1792 cols/core), ReduceScatter of
down-proj partials.

Layout: feature-major activations ([features->partitions, tokens->free]) so
every linear layer uses its natural-layout weight block as the stationary
matmul operand. All activations/weights are fp16 (psum accumulation stays
fp32): halves DMA traffic and doubles DVE element throughput at unchanged PE
rate. The 1/rms factor of rmsnorm1 is applied after the QKV matmul (per-token
column scaling commutes through the contraction); ln weights are folded into
the weight matrices on the host.

Schedule notes (vs the original fp32 version):
- qkv weight chunks are interleaved with the first token block's hb loads so
  the PE starts ~5us in instead of waiting for the full 12.6MB.
- all 6 qkv psum banks are evacuated by immediate scalar-engine copies; the
  rmsnorm/rope chain runs from SBUF off the PE critical path, and the V
  transposes are deferred to a mini-phase after the last block.
- attention processes score tiles in pairs ([128,1024] psum tiles) to halve
  exp/mask/accumulate instruction count; probs in fp16.
- o_proj weights (fp16) stream under the matmul; the first 4 m-tiles and the
  residual slice prefetch during attention.
- the MLP first matmul is single-pass (all 2048 tokens resident in fp16), so
  gate/up weights load once; h round-trips DRAM in fp16 with the phase-5
  reload chunked per k-tile so it overlaps phase 4.
"""

import numpy as np

import concourse.bacc as bacc
import concourse.mybir as mybir
import concourse.tile as tile
from concourse.bass_utils import run_bass_kernel_spmd
from concourse.masks import make_identity

NC = 8
S = 2048
H = 4096
HD = 128
NQ = 32
NKV = 8
I = 14336
QH = NQ // NC        # q heads per core = 4
IPC = I // NC        # intermediate cols per core = 1792
TPC = S // NC        # tokens per core = 256
EPS = 1e-5
THETA = 10000.0
SCALE = HD ** -0.5

F32 = mybir.dt.float32
F16 = mybir.dt.float16

KH = H // 128        # 32 k-tiles over H
NB = S // 512        # 4 token blocks of 512
MB_GU = IPC // 128   # 14 m tiles for gate (and for up)
KI = IPC // 128      # 14 k tiles over I per core

AF = mybir.ActivationFunctionType


def _phase1_qkv(nc, tc, g):
    """QKV matmul + rmsnorm1 stats + rope. Fills qT_sb/kT_sb/vT_sb."""
    with (
        tc.tile_pool(name="p1sbuf", bufs=2) as p1s,
        tc.tile_pool(name="p1psum", bufs=1, space="PSUM") as p1p,
    ):
        tp1 = p1p.tile([128, 8, 128], F16, name="tp1", tag="tp1")
        for nb in range(NB):
            if nb >= 2:
                # V transposes for block nb-2 (data long ready, so the PE
                # passes through them without stalling)
                for t in range(4 * (nb - 2), 4 * (nb - 1)):
                    _vtok_one(nc, tp1, g, t)
            ncols = slice(nb * 512, (nb + 1) * 512)
            st_ps = p1p.tile([1, 512], F32, name="st_ps", tag="st_ps")
            # one psum tile per output block so dependency tracking stays
            # per-bank: the next token block's matmul for slot m waits only
            # on slot m's evacuation copy
            mm_ps = [p1p.tile([128, 512], F32, name=f"mm_ps{m}", tag=f"mm_ps{m}")
                     for m in range(QH + 2)]
            for k in range(KH):
                # weights stream alongside the activations (re-read per
                # block; DMA has slack and this keeps SBUF residency low)
                wqk = p1s.tile([128, (QH + 2) * 128], F16, name="wqk",
                               tag="wqk", bufs=5)
                nc.sync.dma_start(wqk[:], g["wqkv"][:, k, :])
                hb = p1s.tile([128, 512], F16, name="hb", tag="hb", bufs=5)
                nc.sync.dma_start(hb[:], g["hT"][k * 128:(k + 1) * 128, ncols])
                sq = p1s.tile([128, 512], F16, name="sq", tag="sq", bufs=3)
                nc.scalar.activation(sq[:], hb[:], AF.Square)
                nc.tensor.matmul(st_ps[:], g["ones"][:], sq[:],
                                 start=(k == 0), stop=(k == KH - 1))
                for m in range(QH + 2):
                    nc.tensor.matmul(
                        mm_ps[m][:], wqk[:, m * 128:(m + 1) * 128], hb[:],
                        start=(k == 0), stop=(k == KH - 1),
                    )
                if k == 20:
                    # rope tables for this block, needed right after the
                    # k-loop (spread across blocks to keep DMA bursts small)
                    nc.sync.dma_start(g["cos_sb"][:, ncols], g["cosT"][:, ncols])
                    nc.sync.dma_start(g["sin_sb"][:, ncols], g["sinT"][:, ncols])
                    if nb == 0:
                        nc.sync.dma_start(g["mask_sb"][:], g["masks"][:, :])
            # evacuate all 6 psum banks immediately (scalar engine) so the
            # next block's matmuls only wait on these copies, not on the
            # rmsnorm/rope chain below
            qkc = p1s.tile([128, QH + 2, 512], F16, name="qkc", tag="qkc", bufs=1)
            for m in range(QH + 2):
                nc.vector.tensor_copy(qkc[:, m, :], mm_ps[m][:])
            # rstd = exp(-0.5*ln(var+eps)): scalar-engine only, and the
            # ln+exp+square table covers every activation until phase 4
            lnv = p1s.tile([1, 512], F32, name="lnv", tag="lnv")
            nc.scalar.activation(lnv[:], st_ps[:], AF.Ln,
                                 bias=g["epsb"][:], scale=1.0 / H)
            rstd16 = p1s.tile([1, 512], F16, name="rstd16", tag="rstd16")
            nc.scalar.activation(rstd16[:], lnv[:], AF.Exp, scale=-0.5)
            rb = p1s.tile([128, 512], F16, name="rb", tag="rb", bufs=1)
            nc.gpsimd.partition_broadcast(rb[:], rstd16[:])
            cos_s = p1s.tile([128, 512], F16, name="cos_s", tag="cos_s", bufs=1)
            nc.vector.tensor_mul(cos_s[:], g["cos_sb"][:, ncols], rb[:])
            sin_s = p1s.tile([128, 512], F16, name="sin_s", tag="sin_s", bufs=1)
            nc.vector.tensor_mul(sin_s[:], g["sin_sb"][:, ncols], rb[:])
            for m in range(QH + 1):
                if m < QH:
                    d0 = g["qT_sb"][0:64, m, ncols]
                    d1 = g["qT_sb"][64:128, m, ncols]
                else:
                    d0 = g["kT_sb"][0:64, ncols]
                    d1 = g["kT_sb"][64:128, ncols]
                t0 = p1s.tile([64, 512], F16, name="t0", tag="t0", bufs=1)
                nc.vector.tensor_mul(t0[:], qkc[0:64, m, :], cos_s[0:64, :])
                t1 = p1s.tile([64, 512], F16, name="t1", tag="t1", bufs=1)
                nc.vector.tensor_mul(t1[:], qkc[64:128, m, :], sin_s[64:128, :])
                nc.vector.tensor_sub(d0, t0[:], t1[:])
                t2 = p1s.tile([64, 512], F16, name="t2", tag="t0", bufs=1)
                nc.vector.tensor_mul(t2[:], qkc[64:128, m, :], cos_s[64:128, :])
                t3 = p1s.tile([64, 512], F16, name="t3", tag="t1", bufs=1)
                nc.vector.tensor_mul(t3[:], qkc[0:64, m, :], sin_s[0:64, :])
                nc.vector.tensor_add(d1, t2[:], t3[:])
            nc.vector.tensor_mul(g[f"vT{nb}"][:, :], qkc[:, QH + 1, :], rb[:])

        # V transposes for blocks 2-3 happen in phase 2 so their rmsnorm
        # chains never stall the PE at the phase boundary


def _vtok_one(nc, tp, g, t):
    """Transpose one 128-token V tile to token-major for the PV matmuls."""
    nb, j = t // 4, t % 4
    nc.tensor.transpose(tp[:, t % 8, :], g[f"vT{nb}"][:, j * 128:(j + 1) * 128],
                        g["ident"][:])
    nc.vector.tensor_copy(g["v_tok"][:, t, :], tp[:, t % 8, :])


def _phase2_attention(nc, tc, g, with_collectives, rg):
    with (
        tc.tile_pool(name="p2sbuf", bufs=2) as p2s,
        tc.tile_pool(name="p2psum", bufs=1, space="PSUM") as p2p,
    ):
        # prefetch phase-3/4/5 operands with no dependency on attention
        # (phase 2 has plenty of DMA slack)
        nc.sync.dma_start(g["hsl"][:], g["hT_slice"][:, :, :])
        for m in range(3):
            nc.sync.dma_start(g["wo_pre"][:, m, :, :], g["wo"][:, m, :, :])
        nc.sync.dma_start(g["gu_pre"][:], g["wgu"][:, 0, :, :])
        nc.sync.dma_start(g["wdn_pre"][:], g["wdn"][:, 0, :, :])

        for hh in range(QH):
            anob = p2s.tile([128, NB, 512], F16, name="anob", tag="anob", bufs=2)
            for qb in range(NB):
                if hh == 0 and qb == 1:
                    # blocks 2-3's V transposes: emitted after the first score
                    # block so the PE never waits on their rmsnorm chains
                    tp2 = p2p.tile([128, 8, 128], F16, name="tp2", tag="tp2")
                    for t in range(8, 16):
                        _vtok_one(nc, tp2, g, t)
                qcols = slice(qb * 512, (qb + 1) * 512)
                att_ps = p2p.tile([128, 512], F32, name="att_ps", tag="att_ps", bufs=2)
                acc = p2s.tile([128, 2, 512], F16, name="acc", tag="acc", bufs=2)
                # full (unmasked) score tiles, processed in pairs
                for pp in range(2 * qb):
                    kt0, kt1 = 2 * pp, 2 * pp + 1
                    s_ps = p2p.tile([128, 2, 512], F32, name="s_ps", tag="s_ps", bufs=2)
                    nc.tensor.matmul(
                        s_ps[:, 0, :], g["kT_sb"][:, kt0 * 128:(kt0 + 1) * 128],
                        g["qT_sb"][:, hh, qcols], start=True, stop=True,
                    )
                    nc.tensor.matmul(
                        s_ps[:, 1, :], g["kT_sb"][:, kt1 * 128:(kt1 + 1) * 128],
                        g["qT_sb"][:, hh, qcols], start=True, stop=True,
                    )
                    e = p2s.tile([128, 2, 512], F16, name="e", tag="e", bufs=4)
                    nc.scalar.activation(e[:], s_ps[:], AF.Exp, scale=SCALE)
                    if pp == 0:
                        nc.vector.tensor_copy(acc[:], e[:])
                    else:
                        nc.vector.tensor_add(acc[:], acc[:], e[:])
                    nc.tensor.matmul(att_ps[:], g["v_tok"][:, kt0, :], e[:, 0, :],
                                     start=(pp == 0), stop=False)
                    nc.tensor.matmul(att_ps[:], g["v_tok"][:, kt1, :], e[:, 1, :],
                                     start=False, stop=False)
                # the 4 diagonal tiles: causality restricts tile j to local
                # q >= 128j, so matmul/exp/accumulate only the valid width and
                # apply a 128x128 triangle mask to the leading sub-block
                for pj in range(2):
                    s_ps = p2p.tile([128, 2, 512], F32, name="s_ps", tag="s_ps", bufs=2)
                    e = p2s.tile([128, 2, 512], F16, name="e", tag="e", bufs=4)
                    for i in range(2):
                        j = 2 * pj + i
                        kt = 4 * qb + j
                        qoff, w = 128 * j, 512 - 128 * j
                        nc.tensor.matmul(
                            s_ps[:, i, 0:w], g["kT_sb"][:, kt * 128:(kt + 1) * 128],
                            g["qT_sb"][:, hh, qb * 512 + qoff:(qb + 1) * 512],
                            start=True, stop=True,
                        )
                    if pj == 0:
                        # j=0 is full width; j=1 region [384:512) holds stale
                        # psum, exp'd but never read
                        nc.scalar.activation(e[:], s_ps[:], AF.Exp, scale=SCALE)
                    else:
                        nc.scalar.activation(e[:, 0, 0:256], s_ps[:, 0, 0:256],
                                             AF.Exp, scale=SCALE)
                        nc.scalar.activation(e[:, 1, 0:128], s_ps[:, 1, 0:128],
                                             AF.Exp, scale=SCALE)
                    for i in range(2):
                        j = 2 * pj + i
                        kt = 4 * qb + j
                        qoff, w = 128 * j, 512 - 128 * j
                        nc.vector.tensor_mul(e[:, i, 0:128], e[:, i, 0:128],
                                             g["mask_sb"][:, :])
                        if qb == 0 and j == 0:
                            nc.vector.tensor_copy(acc[:, 0, :], e[:, 0, :])
                        elif qb == 0 and j == 1:
                            nc.gpsimd.memset(acc[:, 1, 0:128], 0.0)
                            nc.vector.tensor_copy(acc[:, 1, 128:512], e[:, 1, 0:384])
                        else:
                            nc.vector.tensor_add(acc[:, i, qoff:512],
                                                 acc[:, i, qoff:512], e[:, i, 0:w])
                        nc.tensor.matmul(
                            att_ps[:, qoff:512], g["v_tok"][:, kt, :], e[:, i, 0:w],
                            start=(qb == 0 and j == 0), stop=(j == 3),
                        )
                sums_ps = p2p.tile([1, 512], F32, name="sums_ps", tag="sums_ps", bufs=1)
                nc.tensor.matmul(sums_ps[:], g["ones"][:], acc[:, 0, :],
                                 start=True, stop=False)
                nc.tensor.matmul(sums_ps[:], g["ones"][:], acc[:, 1, :],
                                 start=False, stop=True)
                recip = p2s.tile([1, 512], F32, name="recip", tag="recip")
                nc.vector.reciprocal(recip[:], sums_ps[:])
                recip16 = p2s.tile([1, 512], F16, name="recip16", tag="recip16")
                nc.vector.tensor_copy(recip16[:], recip[:])
                rb2 = p2s.tile([128, 512], F16, name="rb2", tag="rb2", bufs=2)
                nc.gpsimd.partition_broadcast(rb2[:], recip16[:])
                nc.vector.tensor_mul(anob[:, qb, :], att_ps[:], rb2[:])
            # one batched DMA scatters this head's context into the A2A
            # input layout ([core, 128, 256]); per-partition runs stay 512B
            nc.sync.dma_start(
                g[f"a2a_in{hh}"][:, :, :].transpose([1, 0, 2]),
                anob[:, :, :],
            )
            # ship this head's context while the next head computes
            if with_collectives:
                nc.gpsimd.collective_compute(
                    "AllToAll", mybir.AluOpType.bypass, replica_groups=rg,
                    ins=[g[f"a2a_in{hh}"].opt()], outs=[g[f"a2a_out{hh}"].opt()],
                )
                # pull this head's context into the o_proj operand layout
                nc.sync.dma_start(
                    g["asl"][:, hh, :, :],
                    g[f"a2a_out{hh}"][:, :, :].transpose([1, 0, 2]),
                )
            else:
                # local stand-in: same byte count, sourced from SBUF so the
                # o_proj operand isn't chained behind the a2a_in write
                nc.sync.dma_start(g["asl"][:, hh, :, :], anob[:, :, :])


def _phase3_oproj(nc, tc, g, with_collectives, rg):
    with (
        tc.tile_pool(name="p3sbuf", bufs=2) as p3s,
        tc.tile_pool(name="p3psum", bufs=1, space="PSUM") as p3p,
    ):
        res2 = p3s.tile([128, KH, TPC], F16, name="res2", tag="res2", bufs=1)
        st2_ps = p3p.tile([1, TPC], F32, name="st2_ps", tag="st2_ps")
        asl = g["asl"]
        for m in range(KH):
            if m < 3:
                wsrc = g["wo_pre"][:, m]
            else:
                wob = p3s.tile([128, KH, 128], F16, name="wob", tag="wob", bufs=3)
                nc.sync.dma_start(wob[:], g["wo"][:, m, :, :])
                wsrc = wob
            o_ps = p3p.tile([128, TPC], F32, name="o_ps", tag="o_ps", bufs=2)
            for k in range(KH):
                nc.tensor.matmul(o_ps[:], wsrc[:, k, :], asl[:, k // 8, k % 8, :],
                                 start=(k == 0), stop=(k == KH - 1))
            nc.vector.tensor_add(res2[:, m, :], o_ps[:], g["hsl"][:, m, :])
            nc.sync.dma_start(g["res_out"][m * 128:(m + 1) * 128, :], res2[:, m, :])
            sq2 = p3s.tile([128, TPC], F16, name="sq2", tag="sq2", bufs=2)
            nc.vector.tensor_mul(sq2[:], res2[:, m, :], res2[:, m, :])
            nc.tensor.matmul(st2_ps[:], g["ones"][:], sq2[:],
                             start=(m == 0), stop=(m == KH - 1))
        lnv2 = p3s.tile([1, TPC], F32, name="lnv2", tag="lnv2")
        nc.scalar.activation(lnv2[:], st2_ps[:], AF.Ln, bias=g["epsb"][:], scale=1.0 / H)
        rstd2_16 = p3s.tile([1, TPC], F16, name="rstd2_16", tag="rstd2_16")
        nc.scalar.activation(rstd2_16[:], lnv2[:], AF.Exp, scale=-0.5)
        rb3 = p3s.tile([128, TPC], F16, name="rb3", tag="rb3")
        nc.gpsimd.partition_broadcast(rb3[:], rstd2_16[:])
        x2bs = []
        for kq in range(4):
            # batch 8 feature-tiles of normed activations into one DMA, then
            # ship the quarter so phase 4's operands stream in incrementally
            x2b = p3s.tile([128, 8, TPC], F16, name="x2b", tag="x2b", bufs=4)
            x2bs.append(x2b)
            for mi in range(8):
                nc.vector.tensor_mul(x2b[:, mi, :], res2[:, kq * 8 + mi, :], rb3[:])
            if with_collectives:
                nc.sync.dma_start(g[f"ag2_in_q{kq}"][:, :, :], x2b[:, :, :])
                nc.gpsimd.collective_compute(
                    "AllGather", mybir.AluOpType.bypass, replica_groups=rg,
                    ins=[g[f"ag2_in_q{kq}"].opt()],
                    outs=[g[f"ag2_out_q{kq}"].opt()],
                )
        g["x2bs"] = x2bs


def _phase4_gate_up(nc, tc, g, with_collectives):
    """SwiGLU first half, one 512-token quarter at a time.

    Each quarter's activations (4.2MB fp16) double-buffer against the previous
    quarter's compute; gate/up weights re-stream per quarter (DMA has slack).
    h = silu(gate)*up is written straight to the SBUF-resident hful tile, so
    phase 5 starts with everything already on-chip.
    """
    with (
        tc.tile_pool(name="p4x", bufs=1) as p4x,
        tc.tile_pool(name="p4sbuf", bufs=2) as p4s,
        tc.tile_pool(name="p4psum", bufs=1, space="PSUM") as p4p,
    ):
        for tb in range(NB):
            tcols = slice(tb * 512, (tb + 1) * 512)
            if tb % 2 == 0:
                # x2q0 sits in the region freed by the attention pools, so
                # its first-quarter write has no WAR against live phase-3
                # tiles; odd quarters ping-pong through the phase-4 pool
                x2q = g["x2q0"]
            else:
                x2q = p4x.tile([128, KH, 512], F16, name="x2q", tag="x2q", bufs=1)
            if with_collectives or tb > 0:
                for kq in range(4):
                    # one DMA per feature-quarter (both source cores adjacent
                    # in the gather buffer) so the k-loop starts on quarter 0
                    nc.sync.dma_start(
                        x2q[:, kq * 8:(kq + 1) * 8, :],
                        g[f"ag2_out_q{kq}"][2 * tb:2 * tb + 2, :, :, :]
                        .transpose([1, 2, 0, 3]),
                    )
            else:
                # local stand-in for the gather with the same per-core DMA
                # byte count as the real path: this core's slice straight
                # from SBUF, the peer slice from the (unwritten) gather buffer
                for kq in range(4):
                    nc.sync.dma_start(x2q[:, kq * 8:(kq + 1) * 8, 0:256],
                                      g["x2bs"][kq][:, :, :])
                    nc.sync.dma_start(x2q[:, kq * 8:(kq + 1) * 8, 256:512],
                                      g[f"ag2_out_q{kq}"][1, :, :, :])
            if tb == 1 and not with_collectives:
                # collective-input writes (same DMA bytes as the real path's
                # kernel side), deferred behind the first quarters' operands
                for kq in range(4):
                    nc.sync.dma_start(g[f"ag2_in_q{kq}"][:, :, :],
                                      g["x2bs"][kq][:, :, :])
                    nc.sync.dma_start(g[f"ag2_out_q{kq}"][0, :, :, :],
                                      g["x2bs"][kq][:, :, :])
            for m in range(MB_GU):
                if m == 0:
                    gu = g["gu_pre"]
                else:
                    gu = p4s.tile([128, KH, 256], F16, name="gu", tag="gu", bufs=2)
                    nc.sync.dma_start(gu[:], g["wgu"][:, m, :, :])
                g_ps = p4p.tile([128, 512], F32, name="g_ps", tag="g_ps", bufs=2)
                for k in range(KH):
                    nc.tensor.matmul(g_ps[:], gu[:, k, 0:128], x2q[:, k, :],
                                     start=(k == 0), stop=(k == KH - 1))
                u_ps = p4p.tile([128, 512], F32, name="u_ps", tag="u_ps", bufs=2)
                for k in range(KH):
                    nc.tensor.matmul(u_ps[:], gu[:, k, 128:256], x2q[:, k, :],
                                     start=(k == 0), stop=(k == KH - 1))
                sg = p4s.tile([128, 512], F16, name="sg", tag="sg", bufs=2)
                nc.scalar.activation(sg[:], g_ps[:], AF.Silu)
                nc.vector.tensor_mul(g["hful"][:, m, tcols], sg[:], u_ps[:])


def _phase5_down(nc, tc, g, with_collectives, rg):
    with (
        tc.tile_pool(name="p5sbuf", bufs=2) as p5s,
        tc.tile_pool(name="p5psum", bufs=1, space="PSUM") as p5p,
    ):
        hful = g["hful"]
        for r in range(8):
            for mi in range(KH // 8):
                m = r * (KH // 8) + mi
                if m == 0:
                    db = g["wdn_pre"]
                else:
                    db = p5s.tile([128, KI, 128], F16, name="db", tag="db", bufs=3)
                    nc.sync.dma_start(db[:], g["wdn"][:, m, :, :])
                for tb in range(NB):
                    tcols = slice(tb * 512, (tb + 1) * 512)
                    d_ps = p5p.tile([128, 512], F32, name="d_ps", tag="d_ps", bufs=2)
                    for k in range(KI):
                        nc.tensor.matmul(d_ps[:], db[:, k, :], hful[:, k, tcols],
                                         start=(k == 0), stop=(k == KI - 1))
                    ot = p5s.tile([128, 512], F16, name="ot", tag="ot", bufs=2)
                    nc.vector.tensor_copy(ot[:], d_ps[:])
                    nc.sync.dma_start(g[f"rs_in{r}"][mi * 128:(mi + 1) * 128, tcols], ot[:])
            if with_collectives:
                nc.gpsimd.collective_compute(
                    "ReduceScatter", mybir.AluOpType.add, replica_groups=rg,
                    ins=[g[f"rs_in{r}"].opt()], outs=[g[f"rs_out{r}"].opt()],
                )
            else:
                nc.sync.dma_start(g[f"rs_out{r}"][:, :], g[f"rs_in{r}"][0:H // NC // 8, :])
            nc.sync.dma_start(
                g["out_down"][r * 64:(r + 1) * 64, :], g[f"rs_out{r}"][:, :])


def build_program(with_collectives=True, stop_after=99):
    nc = bacc.Bacc("TRN2", target_bir_lowering=False, debug=False, num_devices=NC)

    g = {}
    g["hT"] = nc.dram_tensor("hT", [H, S], F16, kind="ExternalInput")
    g["hT_slice"] = nc.dram_tensor("hT_slice", [128, KH, TPC], F16, kind="ExternalInput")
    g["wqkv"] = nc.dram_tensor("wqkv", [128, KH, (QH + 2) * 128], F16, kind="ExternalInput")
    g["wo"] = nc.dram_tensor("wo", [128, KH, KH, 128], F16, kind="ExternalInput")
    g["wgu"] = nc.dram_tensor("wgu", [128, MB_GU, KH, 256], F16, kind="ExternalInput")
    g["wdn"] = nc.dram_tensor("wdn", [128, KH, KI, 128], F16, kind="ExternalInput")
    g["cosT"] = nc.dram_tensor("cosT", [128, S], F16, kind="ExternalInput")
    g["sinT"] = nc.dram_tensor("sinT", [128, S], F16, kind="ExternalInput")
    g["masks"] = nc.dram_tensor("masks", [128, 128], F16, kind="ExternalInput")

    g["res_out"] = nc.dram_tensor("res_out", [H, TPC], F16, kind="ExternalOutput")
    g["out_down"] = nc.dram_tensor("out_down", [H // NC, S], F16, kind="ExternalOutput")

    rg = [list(range(NC))]

    with tile.TileContext(nc) as tc:
        with (
            tc.tile_pool(name="consts", bufs=1) as consts,
            tc.tile_pool(name="dram", bufs=1, space="DRAM") as dram,
        ):
            for hh in range(QH):
                g[f"a2a_in{hh}"] = dram.tile([NC, 128, TPC], F16, name=f"a2a_in{hh}")
                g[f"a2a_out{hh}"] = dram.tile([NC, 128, TPC], F16, name=f"a2a_out{hh}")
            for kq in range(4):
                g[f"ag2_in_q{kq}"] = dram.tile([128, 8, TPC], F16, name=f"ag2_in_q{kq}")
                g[f"ag2_out_q{kq}"] = dram.tile([NC, 128, 8, TPC], F16,
                                                name=f"ag2_out_q{kq}", addr_space="Shared")
            for r in range(8):
                g[f"rs_in{r}"] = dram.tile([H // 8, S], F16, name=f"rs_in{r}")
                g[f"rs_out{r}"] = dram.tile([H // NC // 8, S], F16, name=f"rs_out{r}")

            ones32 = consts.tile([128, 1], F32, name="ones32")
            nc.gpsimd.memset(ones32[:], 1.0)
            g["ones"] = consts.tile([128, 1], F16, name="ones")
            nc.vector.tensor_copy(g["ones"][:], ones32[:])
            ident32 = consts.tile([128, 128], F32, name="ident32")
            make_identity(nc, ident32[:])
            g["ident"] = consts.tile([128, 128], F16, name="ident")
            nc.vector.tensor_copy(g["ident"][:], ident32[:])
            g["epsb"] = consts.tile([1, 1], F32, name="epsb")
            nc.gpsimd.memset(g["epsb"][:], EPS)
            g["dummy"] = consts.tile([1, 1], F32, name="dummy")
            nc.scalar.add_instruction(mybir.InstLoadActFuncSet(
                name=nc.get_next_instruction_name(), act_func_set_id=6,
                ins=[], outs=[]))

            # h = silu(gate)*up stays SBUF-resident across phases 4 and 5
            with tc.tile_pool(name="mlpkeep", bufs=1) as mlpkeep:
                g["hful"] = mlpkeep.tile([128, KI, S], F16, name="hful")  # 7.3 MB
                g["wdn_pre"] = mlpkeep.tile([128, KI, 128], F16, name="wdn_pre")
                g["gu_pre"] = mlpkeep.tile([128, KH, 256], F16, name="gu_pre")  # 2 MB

                # phase-3 operands that outlive the attention pools
                with tc.tile_pool(name="p3keep", bufs=1) as p3keep:
                    g["hsl"] = p3keep.tile([128, KH, TPC], F16, name="hsl")       # 2 MB
                    g["asl"] = p3keep.tile([128, QH, NC, TPC], F16, name="asl")   # 2 MB
                    g["wo_pre"] = p3keep.tile([128, 3, KH, 128], F16, name="wo_pre")  # 3 MB

                    with tc.tile_pool(name="attn", bufs=1) as attn:
                        g["cos_sb"] = attn.tile([128, S], F16, name="cos_sb")
                        g["sin_sb"] = attn.tile([128, S], F16, name="sin_sb")
                        g["mask_sb"] = attn.tile([128, 128], F16, name="mask_sb")
                        g["qT_sb"] = attn.tile([128, QH, S], F16, name="qT_sb")          # 2 MB
                        g["kT_sb"] = attn.tile([128, S], F16, name="kT_sb")              # 0.5 MB
                        for nb in range(NB):
                            g[f"vT{nb}"] = attn.tile([128, 512], F16, name=f"vT{nb}")
                        g["v_tok"] = attn.tile([128, S // 128, 128], F16, name="v_tok")  # 0.5 MB

                        _phase1_qkv(nc, tc, g)
                        if stop_after >= 2:
                            _phase2_attention(nc, tc, g, with_collectives, rg)

                    if stop_after >= 3:
                        _phase3_oproj(nc, tc, g, with_collectives, rg)

                with tc.tile_pool(name="xq0", bufs=1) as xq0:
                    g["x2q0"] = xq0.tile([128, KH, 512], F16, name="x2q0")  # 4.2 MB

                    if stop_after >= 4:
                        _phase4_gate_up(nc, tc, g, with_collectives)

                if stop_after >= 5:
                    _phase5_down(nc, tc, g, with_collectives, rg)

    nc.finalize()
    return nc


_cached_nc = None


def _get_nc():
    global _cached_nc
    if _cached_nc is None:
        _cached_nc = build_program(with_collectives=True)
    return _cached_nc


def _host_prep(positions, hidden_states, w_qkv, w_o, w_gate_up, w_down, ln1_w, ln2_w):
    f32 = np.float32
    f16 = np.float16
    hidden = np.asarray(hidden_states, dtype=f32)[0]          # [S, H]
    hT = np.ascontiguousarray(hidden.T).astype(f16)            # [H, S]
    pos = np.asarray(positions).astype(f32)[0]                 # [S]

    half = HD // 2
    inv_freq = (1.0 / (f32(THETA) ** (np.arange(0, half, dtype=f32) / f32(half)))).astype(f32)
    ang = pos[:, None] * inv_freq[None, :]                     # [S, 64] fp32
    cos_half = np.cos(ang).astype(f32).T                       # [64, S]
    sin_half = np.sin(ang).astype(f32).T
    cosT_np = np.concatenate([cos_half, cos_half], axis=0).astype(f16)  # [128, S]
    sinT_np = np.concatenate([sin_half, sin_half], axis=0).astype(f16)

    w_qkv_f = np.asarray(w_qkv, dtype=f32) * np.asarray(ln1_w, dtype=f32)[:, None]
    w_gu_f = np.asarray(w_gate_up, dtype=f32) * np.asarray(ln2_w, dtype=f32)[:, None]
    # contraction (k') order is head-major: k' = hh*8 + r <-> global head 4r+hh
    kperm = [4 * (k % NC) + (k // NC) for k in range(KH)]
    w_o_f = np.ascontiguousarray(
        np.asarray(w_o, dtype=f32).reshape(KH, 128, KH, 128)
        .transpose(1, 2, 0, 3)[:, :, kperm, :]
    ).astype(f16)
    w_dn_f = np.asarray(w_down, dtype=f32)

    # causal triangle for the leading 128 columns of each diagonal tile
    masks_np = np.ascontiguousarray(
        (np.arange(128)[None, :] >= np.arange(128)[:, None]).astype(f16))  # [128, 128]

    in_maps = []
    for c in range(NC):
        q_cols = w_qkv_f[:, c * QH * HD:(c + 1) * QH * HD]
        k_col = w_qkv_f[:, NQ * HD + c * HD: NQ * HD + (c + 1) * HD]
        v_col = w_qkv_f[:, (NQ + NKV) * HD + c * HD: (NQ + NKV) * HD + (c + 1) * HD]
        wqkv_c = np.concatenate([q_cols, k_col, v_col], axis=1)
        wqkv_c = np.ascontiguousarray(
            wqkv_c.reshape(KH, 128, (QH + 2) * 128).transpose(1, 0, 2)).astype(f16)
        # per-m interleave: [128, m, k, gate128|up128]
        wg_c = w_gu_f[:, c * IPC:(c + 1) * IPC].reshape(KH, 128, MB_GU, 128)
        wu_c = w_gu_f[:, I + c * IPC: I + (c + 1) * IPC].reshape(KH, 128, MB_GU, 128)
        wgu_c = np.ascontiguousarray(
            np.concatenate([wg_c[..., None, :], wu_c[..., None, :]], axis=3)
            .reshape(KH, 128, MB_GU, 256).transpose(1, 2, 0, 3)).astype(f16)
        wdn_c = np.ascontiguousarray(
            w_dn_f[c * IPC:(c + 1) * IPC, :].reshape(KI, 128, KH, 128)
            .transpose(1, 2, 0, 3)).astype(f16)
        hT_slice_c = np.ascontiguousarray(
            hT[:, c * TPC:(c + 1) * TPC].reshape(KH, 128, TPC).transpose(1, 0, 2))
        in_maps.append({
            "hT": hT,
            "hT_slice": hT_slice_c,
            "wqkv": wqkv_c,
            "wo": w_o_f,
            "wgu": wgu_c,
            "wdn": wdn_c,
            "cosT": cosT_np,
            "sinT": sinT_np,
            "masks": masks_np,
        })
    return in_maps


def kernel(**inputs):
    in_maps = _host_prep(**inputs)
    nc = _get_nc()
    res = run_bass_kernel_spmd(nc, in_maps, core_ids=list(range(NC)))
    results = res.results

    outT = np.empty((H, S), np.float32)
    for c in range(NC):
        od = results[c]["out_down"].astype(np.float32)  # [512, S]
        for r in range(8):
            outT[512 * r + 64 * c: 512 * r + 64 * (c + 1)] = od[64 * r:64 * (r + 1)]
    resT = np.concatenate(
        [results[c]["res_out"].astype(np.float32) for c in range(NC)], axis=1)  # [H, S]
    out = np.ascontiguousarray(outT.T).reshape(1, S, H).astype(np.float32)
    residual = np.ascontiguousarray(resT.T).reshape(1, S, H).astype(np.float32)
    return out, residual


# revision 75
# speedup vs baseline: 1.0852x; 1.0058x over previous
"""Bamba attention decoder layer on 8 Trainium2 NeuronCores.

Sharding: tensor-parallel attention (4 q heads + 1 kv head per core),
AllToAll of attention context (delivers each core its token slice at a static
address), token-sliced o_proj + fused add/rmsnorm, AllGather of normed
activations, I-sharded SwiGLU MLP (1792 cols/core), ReduceScatter of
down-proj partials.

Layout: feature-major activations ([features->partitions, tokens->free]) so
every linear layer uses its natural-layout weight block as the stationary
matmul operand. All activations/weights are fp16 (psum accumulation stays
fp32): halves DMA traffic and doubles DVE element throughput at unchanged PE
rate. The 1/rms factor of rmsnorm1 is applied after the QKV matmul (per-token
column scaling commutes through the contraction); ln weights are folded into
the weight matrices on the host.

Schedule notes (vs the original fp32 version):
- qkv weight chunks are interleaved with the first token block's hb loads so
  the PE starts ~5us in instead of waiting for the full 12.6MB.
- all 6 qkv psum banks are evacuated by immediate scalar-engine copies; the
  rmsnorm/rope chain runs from SBUF off the PE critical path, and the V
  transposes are deferred to a mini-phase after the last block.
- attention processes score tiles in pairs ([128,1024] psum tiles) to halve
  exp/mask/accumulate instruction count; probs in fp16.
- o_proj weights (fp16) stream under the matmul; the first 4 m-tiles and the
  residual slice prefetch during attention.
- the MLP first matmul is single-pass (all 2048 tokens resident in fp16), so
  gate/up weights load once; h round-trips DRAM in fp16 with the phase-5
  reload chunked per k-tile so it overlaps phase 4.
"""

import numpy as np

import concourse.bacc as bacc
import concourse.mybir as mybir
import concourse.tile as tile
from concourse.bass_utils import run_bass_kernel_spmd
from concourse.masks import make_identity

NC = 8
S = 2048
H = 4096
HD = 128
NQ = 32
NKV = 8
I = 14336
QH = NQ // NC        # q heads per core = 4
IPC = I // NC        # intermediate cols per core = 1792
TPC = S // NC        # tokens per core = 256
EPS = 1e-5
THETA = 10000.0
SCALE = HD ** -0.5

F32 = mybir.dt.float32
F16 = mybir.dt.float16

KH = H // 128        # 32 k-tiles over H
NB = S // 512        # 4 token blocks of 512
MB_GU = IPC // 128   # 14 m tiles for gate (and for up)
KI = IPC // 128      # 14 k tiles over I per core

AF = mybir.ActivationFunctionType


def _phase1_qkv(nc, tc, g):
    """QKV matmul + rmsnorm1 stats + rope. Fills qT_sb/kT_sb/vT_sb."""
    with (
        tc.tile_pool(name="p1sbuf", bufs=2) as p1s,
        tc.tile_pool(name="p1psum", bufs=1, space="PSUM") as p1p,
    ):
        tp1 = p1p.tile([128, 8, 128], F16, name="tp1", tag="tp1")
        for nb in range(NB):
            if nb >= 2:
                # V transposes for block nb-2 (data long ready, so the PE
                # passes through them without stalling)
                for t in range(4 * (nb - 2), 4 * (nb - 1)):
                    _vtok_one(nc, tp1, g, t)
            ncols = slice(nb * 512, (nb + 1) * 512)
            st_ps = p1p.tile([1, 512], F32, name="st_ps", tag="st_ps")
            # one psum tile per output block so dependency tracking stays
            # per-bank: the next token block's matmul for slot m waits only
            # on slot m's evacuation copy
            mm_ps = [p1p.tile([128, 512], F32, name=f"mm_ps{m}", tag=f"mm_ps{m}")
                     for m in range(QH + 2)]
            wqk = None
            for k in range(KH):
                # weights stream alongside the activations two k-tiles per
                # DMA (re-read per block; DMA has slack and this keeps SBUF
                # residency low)
                if k % 2 == 0:
                    wqk = p1s.tile([128, 2, (QH + 2) * 128], F16, name="wqk",
                                   tag="wqk", bufs=3)
                    nc.sync.dma_start(wqk[:], g["wqkv"][:, k:k + 2, :])
                hb = p1s.tile([128, 512], F16, name="hb", tag="hb", bufs=5)
                nc.sync.dma_start(hb[:], g["hT"][k * 128:(k + 1) * 128, ncols])
                sq = p1s.tile([128, 512], F16, name="sq", tag="sq", bufs=3)
                nc.scalar.activation(sq[:], hb[:], AF.Square)
                nc.tensor.matmul(st_ps[:], g["ones"][:], sq[:],
                                 start=(k == 0), stop=(k == KH - 1))
                for m in range(QH + 2):
                    nc.tensor.matmul(
                        mm_ps[m][:], wqk[:, k % 2, m * 128:(m + 1) * 128], hb[:],
                        start=(k == 0), stop=(k == KH - 1),
                    )
                if k == 20:
                    # rope tables for this block, needed right after the
                    # k-loop (spread across blocks to keep DMA bursts small)
                    nc.sync.dma_start(g["cos_sb"][:, ncols], g["cosT"][:, ncols])
                    nc.sync.dma_start(g["sin_sb"][:, ncols], g["sinT"][:, ncols])
                    if nb == 0:
                        nc.sync.dma_start(g["mask_sb"][:], g["masks"][:, :])
            # evacuate all 6 psum banks immediately (scalar engine) so the
            # next block's matmuls only wait on these copies, not on the
            # rmsnorm/rope chain below
            qkc = p1s.tile([128, QH + 2, 512], F16, name="qkc", tag="qkc", bufs=1)
            for m in range(QH + 2):
                nc.vector.tensor_copy(qkc[:, m, :], mm_ps[m][:])
            # rstd = exp(-0.5*ln(var+eps)): scalar-engine only, and the
            # ln+exp+square table covers every activation until phase 4
            lnv = p1s.tile([1, 512], F32, name="lnv", tag="lnv")
            nc.scalar.activation(lnv[:], st_ps[:], AF.Ln,
                                 bias=g["epsb"][:], scale=1.0 / H)
            rstd16 = p1s.tile([1, 512], F16, name="rstd16", tag="rstd16")
            nc.scalar.activation(rstd16[:], lnv[:], AF.Exp, scale=-0.5)
            rb = p1s.tile([128, 512], F16, name="rb", tag="rb", bufs=1)
            nc.gpsimd.partition_broadcast(rb[:], rstd16[:])
            cos_s = p1s.tile([128, 512], F16, name="cos_s", tag="cos_s", bufs=1)
            nc.vector.tensor_mul(cos_s[:], g["cos_sb"][:, ncols], rb[:])
            sin_s = p1s.tile([128, 512], F16, name="sin_s", tag="sin_s", bufs=1)
            nc.vector.tensor_mul(sin_s[:], g["sin_sb"][:, ncols], rb[:])
            for m in range(QH + 1):
                if m < QH:
                    d0 = g[f"qT{nb}"][0:64, m, :]
                    d1 = g[f"qT{nb}"][64:128, m, :]
                else:
                    d0 = g[f"kT{nb}"][0:64, :]
                    d1 = g[f"kT{nb}"][64:128, :]
                t0 = p1s.tile([64, 512], F16, name="t0", tag="t0", bufs=1)
                nc.vector.tensor_mul(t0[:], qkc[0:64, m, :], cos_s[0:64, :])
                t1 = p1s.tile([64, 512], F16, name="t1", tag="t1", bufs=1)
                nc.vector.tensor_mul(t1[:], qkc[64:128, m, :], sin_s[64:128, :])
                nc.vector.tensor_sub(d0, t0[:], t1[:])
                t2 = p1s.tile([64, 512], F16, name="t2", tag="t0", bufs=1)
                nc.vector.tensor_mul(t2[:], qkc[64:128, m, :], cos_s[64:128, :])
                t3 = p1s.tile([64, 512], F16, name="t3", tag="t1", bufs=1)
                nc.vector.tensor_mul(t3[:], qkc[0:64, m, :], sin_s[0:64, :])
                nc.vector.tensor_add(d1, t2[:], t3[:])
            nc.vector.tensor_mul(g[f"vT{nb}"][:, :], qkc[:, QH + 1, :], rb[:])

        # V transposes for blocks 2-3 happen in phase 2 so their rmsnorm
        # chains never stall the PE at the phase boundary


def _vtok_one(nc, tp, g, t):
    """Transpose one 128-token V tile to token-major for the PV matmuls."""
    nb, j = t // 4, t % 4
    nc.tensor.transpose(tp[:, t % 8, :], g[f"vT{nb}"][:, j * 128:(j + 1) * 128],
                        g["ident"][:])
    nc.vector.tensor_copy(g["v_tok"][:, t, :], tp[:, t % 8, :])


def _phase2_attention(nc, tc, g, with_collectives, rg):
    with (
        tc.tile_pool(name="p2sbuf", bufs=2) as p2s,
        tc.tile_pool(name="p2psum", bufs=1, space="PSUM") as p2p,
    ):
        # prefetch phase-3/4/5 operands with no dependency on attention
        # (phase 2 has plenty of DMA slack)
        nc.sync.dma_start(g["hsl"][:], g["hT_slice"][:, :, :])
        for m in range(3):
            nc.sync.dma_start(g["wo_pre"][:, m, :, :], g["wo"][:, m, :, :])
        nc.sync.dma_start(g["gu_pre"][:], g["wgu"][:, 0, :, :])
        nc.sync.dma_start(g["wdn_pre"][:], g["wdn"][:, 0, :, :])

        for hh in range(QH):
            anob = p2s.tile([128, NB, 512], F16, name="anob", tag="anob", bufs=2)
            for qb in range(NB):
                if hh == 0 and qb in (1, 2):
                    # blocks 2/3's V transposes: emitted one block before
                    # their first use so the DVE copies complete in time
                    tp2 = p2p.tile([128, 8, 128], F16, name="tp2", tag="tp2")
                    for t in range(4 + 4 * qb, 8 + 4 * qb):
                        _vtok_one(nc, tp2, g, t)
                qcols = slice(qb * 512, (qb + 1) * 512)
                att_ps = p2p.tile([128, 512], F32, name="att_ps", tag="att_ps", bufs=2)
                acc = p2s.tile([128, 2, 512], F16, name="acc", tag="acc", bufs=2)
                # full (unmasked) score tiles, processed in pairs
                for pp in range(2 * qb):
                    kt0, kt1 = 2 * pp, 2 * pp + 1
                    s_ps = p2p.tile([128, 2, 512], F32, name="s_ps", tag="s_ps", bufs=2)
                    nc.tensor.matmul(
                        s_ps[:, 0, :],
                        g[f"kT{kt0 // 4}"][:, (kt0 % 4) * 128:(kt0 % 4 + 1) * 128],
                        g[f"qT{qb}"][:, hh, :], start=True, stop=True,
                    )
                    nc.tensor.matmul(
                        s_ps[:, 1, :],
                        g[f"kT{kt1 // 4}"][:, (kt1 % 4) * 128:(kt1 % 4 + 1) * 128],
                        g[f"qT{qb}"][:, hh, :], start=True, stop=True,
                    )
                    e = p2s.tile([128, 2, 512], F16, name="e", tag="e", bufs=4)
                    nc.scalar.activation(e[:], s_ps[:], AF.Exp, scale=SCALE)
                    if pp == 0:
                        nc.vector.tensor_copy(acc[:], e[:])
                    else:
                        nc.vector.tensor_add(acc[:], acc[:], e[:])
                    nc.tensor.matmul(att_ps[:], g["v_tok"][:, kt0, :], e[:, 0, :],
                                     start=(pp == 0), stop=False)
                    nc.tensor.matmul(att_ps[:], g["v_tok"][:, kt1, :], e[:, 1, :],
                                     start=False, stop=False)
                # the 4 diagonal tiles: causality restricts tile j to local
                # q >= 128j, so matmul/exp/accumulate only the valid width and
                # apply a 128x128 triangle mask to the leading sub-block
                for pj in range(2):
                    s_ps = p2p.tile([128, 2, 512], F32, name="s_ps", tag="s_ps", bufs=2)
                    e = p2s.tile([128, 2, 512], F16, name="e", tag="e", bufs=4)
                    for i in range(2):
                        j = 2 * pj + i
                        kt = 4 * qb + j
                        qoff, w = 128 * j, 512 - 128 * j
                        nc.tensor.matmul(
                            s_ps[:, i, 0:w],
                            g[f"kT{qb}"][:, j * 128:(j + 1) * 128],
                            g[f"qT{qb}"][:, hh, qoff:512],
                            start=True, stop=True,
                        )
                    if pj == 0:
                        # j=0 is full width; j=1 region [384:512) holds stale
                        # psum, exp'd but never read
                        nc.scalar.activation(e[:], s_ps[:], AF.Exp, scale=SCALE)
                    else:
                        nc.scalar.activation(e[:, 0, 0:256], s_ps[:, 0, 0:256],
                                             AF.Exp, scale=SCALE)
                        nc.scalar.activation(e[:, 1, 0:128], s_ps[:, 1, 0:128],
                                             AF.Exp, scale=SCALE)
                    for i in range(2):
                        j = 2 * pj + i
                        kt = 4 * qb + j
                        qoff, w = 128 * j, 512 - 128 * j
                        nc.vector.tensor_mul(e[:, i, 0:128], e[:, i, 0:128],
                                             g["mask_sb"][:, :])
                        if qb == 0 and j == 0:
                            nc.vector.tensor_copy(acc[:, 0, :], e[:, 0, :])
                        elif qb == 0 and j == 1:
                            nc.gpsimd.memset(acc[:, 1, 0:128], 0.0)
                            nc.vector.tensor_copy(acc[:, 1, 128:512], e[:, 1, 0:384])
                        else:
                            nc.vector.tensor_add(acc[:, i, qoff:512],
                                                 acc[:, i, qoff:512], e[:, i, 0:w])
                        nc.tensor.matmul(
                            att_ps[:, qoff:512], g["v_tok"][:, kt, :], e[:, i, 0:w],
                            start=(qb == 0 and j == 0), stop=(j == 3),
                        )
                sums_ps = p2p.tile([1, 512], F32, name="sums_ps", tag="sums_ps", bufs=1)
                nc.tensor.matmul(sums_ps[:], g["ones"][:], acc[:, 0, :],
                                 start=True, stop=False)
                nc.tensor.matmul(sums_ps[:], g["ones"][:], acc[:, 1, :],
                                 start=False, stop=True)
                recip = p2s.tile([1, 512], F32, name="recip", tag="recip")
                nc.vector.reciprocal(recip[:], sums_ps[:])
                recip16 = p2s.tile([1, 512], F16, name="recip16", tag="recip16")
                nc.vector.tensor_copy(recip16[:], recip[:])
                rb2 = p2s.tile([128, 512], F16, name="rb2", tag="rb2", bufs=2)
                nc.gpsimd.partition_broadcast(rb2[:], recip16[:])
                nc.vector.tensor_mul(anob[:, qb, :], att_ps[:], rb2[:])
            # one batched DMA scatters this head's context into the A2A
            # input layout ([core, 128, 256]); per-partition runs stay 512B
            nc.sync.dma_start(
                g[f"a2a_in{hh}"][:, :, :].transpose([1, 0, 2]),
                anob[:, :, :],
            )
            # ship this head's context while the next head computes
            if with_collectives:
                nc.gpsimd.collective_compute(
                    "AllToAll", mybir.AluOpType.bypass, replica_groups=rg,
                    ins=[g[f"a2a_in{hh}"].opt()], outs=[g[f"a2a_out{hh}"].opt()],
                )
                # pull this head's context into the o_proj operand layout
                nc.sync.dma_start(
                    g["asl"][:, hh, :, :],
                    g[f"a2a_out{hh}"][:, :, :].transpose([1, 0, 2]),
                )
            else:
                # local stand-in: same byte count, sourced from SBUF so the
                # o_proj operand isn't chained behind the a2a_in write
                nc.sync.dma_start(g["asl"][:, hh, :, :], anob[:, :, :])


def _phase3_oproj(nc, tc, g, with_collectives, rg):
    with (
        tc.tile_pool(name="p3sbuf", bufs=2) as p3s,
        tc.tile_pool(name="p3psum", bufs=1, space="PSUM") as p3p,
    ):
        res2 = p3s.tile([128, KH, TPC], F16, name="res2", tag="res2", bufs=1)
        st2_ps = p3p.tile([1, TPC], F32, name="st2_ps", tag="st2_ps")
        asl = g["asl"]
        for m in range(KH):
            if m < 3:
                wsrc = g["wo_pre"][:, m]
            else:
                wob = p3s.tile([128, KH, 128], F16, name="wob", tag="wob", bufs=3)
                nc.sync.dma_start(wob[:], g["wo"][:, m, :, :])
                wsrc = wob
            o_ps = p3p.tile([128, TPC], F32, name="o_ps", tag="o_ps", bufs=2)
            for k in range(KH):
                nc.tensor.matmul(o_ps[:], wsrc[:, k, :], asl[:, k // 8, k % 8, :],
                                 start=(k == 0), stop=(k == KH - 1))
            nc.vector.tensor_add(res2[:, m, :], o_ps[:], g["hsl"][:, m, :])
            nc.sync.dma_start(g["res_out"][m * 128:(m + 1) * 128, :], res2[:, m, :])
            sq2 = p3s.tile([128, TPC], F16, name="sq2", tag="sq2", bufs=2)
            nc.vector.tensor_mul(sq2[:], res2[:, m, :], res2[:, m, :])
            nc.tensor.matmul(st2_ps[:], g["ones"][:], sq2[:],
                             start=(m == 0), stop=(m == KH - 1))
        lnv2 = p3s.tile([1, TPC], F32, name="lnv2", tag="lnv2")
        nc.scalar.activation(lnv2[:], st2_ps[:], AF.Ln, bias=g["epsb"][:], scale=1.0 / H)
        rstd2_16 = p3s.tile([1, TPC], F16, name="rstd2_16", tag="rstd2_16")
        nc.scalar.activation(rstd2_16[:], lnv2[:], AF.Exp, scale=-0.5)
        rb3 = p3s.tile([128, TPC], F16, name="rb3", tag="rb3")
        nc.gpsimd.partition_broadcast(rb3[:], rstd2_16[:])
        x2bs = []
        for kq in range(4):
            # batch 8 feature-tiles of normed activations into one DMA, then
            # ship the quarter so phase 4's operands stream in incrementally
            x2b = p3s.tile([128, 8, TPC], F16, name="x2b", tag="x2b", bufs=4)
            x2bs.append(x2b)
            for mi in range(8):
                nc.vector.tensor_mul(x2b[:, mi, :], res2[:, kq * 8 + mi, :], rb3[:])
            if with_collectives:
                nc.sync.dma_start(g[f"ag2_in_q{kq}"][:, :, :], x2b[:, :, :])
                nc.gpsimd.collective_compute(
                    "AllGather", mybir.AluOpType.bypass, replica_groups=rg,
                    ins=[g[f"ag2_in_q{kq}"].opt()],
                    outs=[g[f"ag2_out_q{kq}"].opt()],
                )
        g["x2bs"] = x2bs


def _phase4_gate_up(nc, tc, g, with_collectives):
    """SwiGLU first half, one 512-token quarter at a time.

    Each quarter's activations (4.2MB fp16) double-buffer against the previous
    quarter's compute; gate/up weights re-stream per quarter (DMA has slack).
    h = silu(gate)*up is written straight to the SBUF-resident hful tile, so
    phase 5 starts with everything already on-chip.
    """
    with (
        tc.tile_pool(name="p4x", bufs=1) as p4x,
        tc.tile_pool(name="p4sbuf", bufs=2) as p4s,
        tc.tile_pool(name="p4psum", bufs=1, space="PSUM") as p4p,
    ):
        for tb in range(NB):
            tcols = slice(tb * 512, (tb + 1) * 512)
            if tb % 2 == 0:
                # x2q0 sits in the region freed by the attention pools, so
                # its first-quarter write has no WAR against live phase-3
                # tiles; odd quarters ping-pong through the phase-4 pool
                x2q = g["x2q0"]
            else:
                x2q = p4x.tile([128, KH, 512], F16, name="x2q", tag="x2q", bufs=1)
            if with_collectives or tb > 0:
                for kq in range(4):
                    # one DMA per feature-quarter (both source cores adjacent
                    # in the gather buffer) so the k-loop starts on quarter 0
                    nc.sync.dma_start(
                        x2q[:, kq * 8:(kq + 1) * 8, :],
                        g[f"ag2_out_q{kq}"][2 * tb:2 * tb + 2, :, :, :]
                        .transpose([1, 2, 0, 3]),
                    )
            else:
                # local stand-in for the gather with the same per-core DMA
                # byte count as the real path: this core's slice straight
                # from SBUF, the peer slice from the (unwritten) gather buffer
                for kq in range(4):
                    nc.sync.dma_start(x2q[:, kq * 8:(kq + 1) * 8, 0:256],
                                      g["x2bs"][kq][:, :, :])
                    nc.sync.dma_start(x2q[:, kq * 8:(kq + 1) * 8, 256:512],
                                      g[f"ag2_out_q{kq}"][1, :, :, :])
            if tb == 1 and not with_collectives:
                # collective-input writes (same DMA bytes as the real path's
                # kernel side), deferred behind the first quarters' operands
                for kq in range(4):
                    nc.sync.dma_start(g[f"ag2_in_q{kq}"][:, :, :],
                                      g["x2bs"][kq][:, :, :])
                    nc.sync.dma_start(g[f"ag2_out_q{kq}"][0, :, :, :],
                                      g["x2bs"][kq][:, :, :])
            for m in range(MB_GU):
                if m == 0:
                    gu = g["gu_pre"]
                else:
                    gu = p4s.tile([128, KH, 256], F16, name="gu", tag="gu", bufs=2)
                    nc.sync.dma_start(gu[:], g["wgu"][:, m, :, :])
                g_ps = p4p.tile([128, 512], F32, name="g_ps", tag="g_ps", bufs=2)
                for k in range(KH):
                    nc.tensor.matmul(g_ps[:], gu[:, k, 0:128], x2q[:, k, :],
                                     start=(k == 0), stop=(k == KH - 1))
                u_ps = p4p.tile([128, 512], F32, name="u_ps", tag="u_ps", bufs=2)
                for k in range(KH):
                    nc.tensor.matmul(u_ps[:], gu[:, k, 128:256], x2q[:, k, :],
                                     start=(k == 0), stop=(k == KH - 1))
                sg = p4s.tile([128, 512], F16, name="sg", tag="sg", bufs=2)
                nc.scalar.activation(sg[:], g_ps[:], AF.Silu)
                nc.vector.tensor_mul(g["hful"][:, m, tcols], sg[:], u_ps[:])


def _phase5_down(nc, tc, g, with_collectives, rg):
    with (
        tc.tile_pool(name="p5sbuf", bufs=2) as p5s,
        tc.tile_pool(name="p5psum", bufs=1, space="PSUM") as p5p,
    ):
        hful = g["hful"]
        for r in range(8):
            for mi in range(KH // 8):
                m = r * (KH // 8) + mi
                if m == 0:
                    db = g["wdn_pre"]
                else:
                    db = p5s.tile([128, KI, 128], F16, name="db", tag="db", bufs=3)
                    nc.sync.dma_start(db[:], g["wdn"][:, m, :, :])
                for tb in range(NB):
                    tcols = slice(tb * 512, (tb + 1) * 512)
                    d_ps = p5p.tile([128, 512], F32, name="d_ps", tag="d_ps", bufs=2)
                    for k in range(KI):
                        nc.tensor.matmul(d_ps[:], db[:, k, :], hful[:, k, tcols],
                                         start=(k == 0), stop=(k == KI - 1))
                    ot = p5s.tile([128, 512], F16, name="ot", tag="ot", bufs=2)
                    nc.vector.tensor_copy(ot[:], d_ps[:])
                    nc.sync.dma_start(g[f"rs_in{r}"][mi * 128:(mi + 1) * 128, tcols], ot[:])
            if with_collectives:
                nc.gpsimd.collective_compute(
                    "ReduceScatter", mybir.AluOpType.add, replica_groups=rg,
                    ins=[g[f"rs_in{r}"].opt()], outs=[g[f"rs_out{r}"].opt()],
                )
            else:
                nc.sync.dma_start(g[f"rs_out{r}"][:, :], g[f"rs_in{r}"][0:H // NC // 8, :])
            nc.sync.dma_start(
                g["out_down"][r * 64:(r + 1) * 64, :], g[f"rs_out{r}"][:, :])


def build_program(with_collectives=True, stop_after=99):
    nc = bacc.Bacc("TRN2", target_bir_lowering=False, debug=False, num_devices=NC)

    g = {}
    g["hT"] = nc.dram_tensor("hT", [H, S], F16, kind="ExternalInput")
    g["hT_slice"] = nc.dram_tensor("hT_slice", [128, KH, TPC], F16, kind="ExternalInput")
    g["wqkv"] = nc.dram_tensor("wqkv", [128, KH, (QH + 2) * 128], F16, kind="ExternalInput")
    g["wo"] = nc.dram_tensor("wo", [128, KH, KH, 128], F16, kind="ExternalInput")
    g["wgu"] = nc.dram_tensor("wgu", [128, MB_GU, KH, 256], F16, kind="ExternalInput")
    g["wdn"] = nc.dram_tensor("wdn", [128, KH, KI, 128], F16, kind="ExternalInput")
    g["cosT"] = nc.dram_tensor("cosT", [128, S], F16, kind="ExternalInput")
    g["sinT"] = nc.dram_tensor("sinT", [128, S], F16, kind="ExternalInput")
    g["masks"] = nc.dram_tensor("masks", [128, 128], F16, kind="ExternalInput")

    g["res_out"] = nc.dram_tensor("res_out", [H, TPC], F16, kind="ExternalOutput")
    g["out_down"] = nc.dram_tensor("out_down", [H // NC, S], F16, kind="ExternalOutput")

    rg = [list(range(NC))]

    with tile.TileContext(nc) as tc:
        with (
            tc.tile_pool(name="consts", bufs=1) as consts,
            tc.tile_pool(name="dram", bufs=1, space="DRAM") as dram,
        ):
            for hh in range(QH):
                g[f"a2a_in{hh}"] = dram.tile([NC, 128, TPC], F16, name=f"a2a_in{hh}")
                g[f"a2a_out{hh}"] = dram.tile([NC, 128, TPC], F16, name=f"a2a_out{hh}")
            for kq in range(4):
                g[f"ag2_in_q{kq}"] = dram.tile([128, 8, TPC], F16, name=f"ag2_in_q{kq}")
                g[f"ag2_out_q{kq}"] = dram.tile([NC, 128, 8, TPC], F16,
                                                name=f"ag2_out_q{kq}", addr_space="Shared")
            for r in range(8):
                g[f"rs_in{r}"] = dram.tile([H // 8, S], F16, name=f"rs_in{r}")
                g[f"rs_out{r}"] = dram.tile([H // NC // 8, S], F16, name=f"rs_out{r}")

            ones32 = consts.tile([128, 1], F32, name="ones32")
            nc.gpsimd.memset(ones32[:], 1.0)
            g["ones"] = consts.tile([128, 1], F16, name="ones")
            nc.vector.tensor_copy(g["ones"][:], ones32[:])
            ident32 = consts.tile([128, 128], F32, name="ident32")
            make_identity(nc, ident32[:])
            g["ident"] = consts.tile([128, 128], F16, name="ident")
            nc.vector.tensor_copy(g["ident"][:], ident32[:])
            g["epsb"] = consts.tile([1, 1], F32, name="epsb")
            nc.gpsimd.memset(g["epsb"][:], EPS)
            g["dummy"] = consts.tile([1, 1], F32, name="dummy")
            nc.scalar.add_instruction(mybir.InstLoadActFuncSet(
                name=nc.get_next_instruction_name(), act_func_set_id=6,
                ins=[], outs=[]))

            # h = silu(gate)*up stays SBUF-resident across phases 4 and 5
            with tc.tile_pool(name="mlpkeep", bufs=1) as mlpkeep:
                g["hful"] = mlpkeep.tile([128, KI, S], F16, name="hful")  # 7.3 MB
                g["wdn_pre"] = mlpkeep.tile([128, KI, 128], F16, name="wdn_pre")
                g["gu_pre"] = mlpkeep.tile([128, KH, 256], F16, name="gu_pre")  # 2 MB

                # phase-3 operands that outlive the attention pools
                with tc.tile_pool(name="p3keep", bufs=1) as p3keep:
                    g["hsl"] = p3keep.tile([128, KH, TPC], F16, name="hsl")       # 2 MB
                    g["asl"] = p3keep.tile([128, QH, NC, TPC], F16, name="asl")   # 2 MB
                    g["wo_pre"] = p3keep.tile([128, 3, KH, 128], F16, name="wo_pre")  # 3 MB

                    with tc.tile_pool(name="attn", bufs=1) as attn:
                        g["cos_sb"] = attn.tile([128, S], F16, name="cos_sb")
                        g["sin_sb"] = attn.tile([128, S], F16, name="sin_sb")
                        g["mask_sb"] = attn.tile([128, 128], F16, name="mask_sb")
                        for nb in range(NB):
                            g[f"qT{nb}"] = attn.tile([128, QH, 512], F16, name=f"qT{nb}")
                            g[f"kT{nb}"] = attn.tile([128, 512], F16, name=f"kT{nb}")
                        for nb in range(NB):
                            g[f"vT{nb}"] = attn.tile([128, 512], F16, name=f"vT{nb}")
                        g["v_tok"] = attn.tile([128, S // 128, 128], F16, name="v_tok")  # 0.5 MB

                        _phase1_qkv(nc, tc, g)
                        if stop_after >= 2:
                            _phase2_attention(nc, tc, g, with_collectives, rg)

                    if stop_after >= 3:
                        _phase3_oproj(nc, tc, g, with_collectives, rg)

                with tc.tile_pool(name="xq0", bufs=1) as xq0:
                    g["x2q0"] = xq0.tile([128, KH, 512], F16, name="x2q0")  # 4.2 MB

                    if stop_after >= 4:
                        _phase4_gate_up(nc, tc, g, with_collectives)

                if stop_after >= 5:
                    _phase5_down(nc, tc, g, with_collectives, rg)

    nc.finalize()
    return nc


_cached_nc = None


def _get_nc():
    global _cached_nc
    if _cached_nc is None:
        _cached_nc = build_program(with_collectives=True)
    return _cached_nc


def _host_prep(positions, hidden_states, w_qkv, w_o, w_gate_up, w_down, ln1_w, ln2_w):
    f32 = np.float32
    f16 = np.float16
    hidden = np.asarray(hidden_states, dtype=f32)[0]          # [S, H]
    hT = np.ascontiguousarray(hidden.T).astype(f16)            # [H, S]
    pos = np.asarray(positions).astype(f32)[0]                 # [S]

    half = HD // 2
    inv_freq = (1.0 / (f32(THETA) ** (np.arange(0, half, dtype=f32) / f32(half)))).astype(f32)
    ang = pos[:, None] * inv_freq[None, :]                     # [S, 64] fp32
    cos_half = np.cos(ang).astype(f32).T                       # [64, S]
    sin_half = np.sin(ang).astype(f32).T
    cosT_np = np.concatenate([cos_half, cos_half], axis=0).astype(f16)  # [128, S]
    sinT_np = np.concatenate([sin_half, sin_half], axis=0).astype(f16)

    w_qkv_f = np.asarray(w_qkv, dtype=f32) * np.asarray(ln1_w, dtype=f32)[:, None]
    w_gu_f = np.asarray(w_gate_up, dtype=f32) * np.asarray(ln2_w, dtype=f32)[:, None]
    # contraction (k') order is head-major: k' = hh*8 + r <-> global head 4r+hh
    kperm = [4 * (k % NC) + (k // NC) for k in range(KH)]
    w_o_f = np.ascontiguousarray(
        np.asarray(w_o, dtype=f32).reshape(KH, 128, KH, 128)
        .transpose(1, 2, 0, 3)[:, :, kperm, :]
    ).astype(f16)
    w_dn_f = np.asarray(w_down, dtype=f32)

    # causal triangle for the leading 128 columns of each diagonal tile
    masks_np = np.ascontiguousarray(
        (np.arange(128)[None, :] >= np.arange(128)[:, None]).astype(f16))  # [128, 128]

    in_maps = []
    for c in range(NC):
        q_cols = w_qkv_f[:, c * QH * HD:(c + 1) * QH * HD]
        k_col = w_qkv_f[:, NQ * HD + c * HD: NQ * HD + (c + 1) * HD]
        v_col = w_qkv_f[:, (NQ + NKV) * HD + c * HD: (NQ + NKV) * HD + (c + 1) * HD]
        wqkv_c = np.concatenate([q_cols, k_col, v_col], axis=1)
        wqkv_c = np.ascontiguousarray(
            wqkv_c.reshape(KH, 128, (QH + 2) * 128).transpose(1, 0, 2)).astype(f16)
        # per-m interleave: [128, m, k, gate128|up128]
        wg_c = w_gu_f[:, c * IPC:(c + 1) * IPC].reshape(KH, 128, MB_GU, 128)
        wu_c = w_gu_f[:, I + c * IPC: I + (c + 1) * IPC].reshape(KH, 128, MB_GU, 128)
        wgu_c = np.ascontiguousarray(
            np.concatenate([wg_c[..., None, :], wu_c[..., None, :]], axis=3)
            .reshape(KH, 128, MB_GU, 256).transpose(1, 2, 0, 3)).astype(f16)
        wdn_c = np.ascontiguousarray(
            w_dn_f[c * IPC:(c + 1) * IPC, :].reshape(KI, 128, KH, 128)
            .transpose(1, 2, 0, 3)).astype(f16)
        hT_slice_c = np.ascontiguousarray(
            hT[:, c * TPC:(c + 1) * TPC].reshape(KH, 128, TPC).transpose(1, 0, 2))
        in_maps.append({
            "hT": hT,
            "hT_slice": hT_slice_c,
            "wqkv": wqkv_c,
            "wo": w_o_f,
            "wgu": wgu_c,
            "wdn": wdn_c,
            "cosT": cosT_np,
            "sinT": sinT_np,
            "masks": masks_np,
        })
    return in_maps


def kernel(**inputs):
    in_maps = _host_prep(**inputs)
    nc = _get_nc()
    res = run_bass_kernel_spmd(nc, in_maps, core_ids=list(range(NC)))
    results = res.results

    outT = np.empty((H, S), np.float32)
    for c in range(NC):
        od = results[c]["out_down"].astype(np.float32)  # [512, S]
        for r in range(8):
            outT[512 * r + 64 * c: 512 * r + 64 * (c + 1)] = od[64 * r:64 * (r + 1)]
    resT = np.concatenate(
        [results[c]["res_out"].astype(np.float32) for c in range(NC)], axis=1)  # [H, S]
    out = np.ascontiguousarray(outT.T).reshape(1, S, H).astype(np.float32)
    residual = np.ascontiguousarray(resT.T).reshape(1, S, H).astype(np.float32)
    return out, residual


# revision 90
# speedup vs baseline: 1.0869x; 1.0016x over previous
"""Bamba attention decoder layer on 8 Trainium2 NeuronCores.

Sharding: tensor-parallel attention (4 q heads + 1 kv head per core),
AllToAll of attention context (delivers each core its token slice at a static
address), token-sliced o_proj + fused add/rmsnorm, AllGather of normed
activations, I-sharded SwiGLU MLP (1792 cols/core), ReduceScatter of
down-proj partials.

Layout: feature-major activations ([features->partitions, tokens->free]) so
every linear layer uses its natural-layout weight block as the stationary
matmul operand. All activations/weights are fp16 (psum accumulation stays
fp32): halves DMA traffic and doubles DVE element throughput at unchanged PE
rate. The 1/rms factor of rmsnorm1 is applied after the QKV matmul (per-token
column scaling commutes through the contraction); ln weights are folded into
the weight matrices on the host.

Schedule notes (vs the original fp32 version):
- qkv weight chunks are interleaved with the first token block's hb loads so
  the PE starts ~5us in instead of waiting for the full 12.6MB.
- all 6 qkv psum banks are evacuated by immediate scalar-engine copies; the
  rmsnorm/rope chain runs from SBUF off the PE critical path, and the V
  transposes are deferred to a mini-phase after the last block.
- attention processes score tiles in pairs ([128,1024] psum tiles) to halve
  exp/mask/accumulate instruction count; probs in fp16.
- o_proj weights (fp16) stream under the matmul; the first 4 m-tiles and the
  residual slice prefetch during attention.
- the MLP first matmul is single-pass (all 2048 tokens resident in fp16), so
  gate/up weights load once; h round-trips DRAM in fp16 with the phase-5
  reload chunked per k-tile so it overlaps phase 4.
"""

import numpy as np

import concourse.bacc as bacc
import concourse.mybir as mybir
import concourse.tile as tile
from concourse.bass_utils import run_bass_kernel_spmd
from concourse.masks import make_identity

NC = 8
S = 2048
H = 4096
HD = 128
NQ = 32
NKV = 8
I = 14336
QH = NQ // NC        # q heads per core = 4
IPC = I // NC        # intermediate cols per core = 1792
TPC = S // NC        # tokens per core = 256
EPS = 1e-5
THETA = 10000.0
SCALE = HD ** -0.5

F32 = mybir.dt.float32
F16 = mybir.dt.float16

KH = H // 128        # 32 k-tiles over H
NB = S // 512        # 4 token blocks of 512
MB_GU = IPC // 128   # 14 m tiles for gate (and for up)
KI = IPC // 128      # 14 k tiles over I per core

AF = mybir.ActivationFunctionType


def _phase1_qkv(nc, tc, g):
    """QKV matmul + rmsnorm1 stats + rope. Fills qT_sb/kT_sb/vT_sb."""
    with (
        tc.tile_pool(name="p1sbuf", bufs=2) as p1s,
        tc.tile_pool(name="p1psum", bufs=1, space="PSUM") as p1p,
    ):
        tp1 = p1p.tile([128, 8, 128], F16, name="tp1", tag="tp1")
        for nb in range(NB):
            if nb >= 2:
                # V transposes for block nb-2 (data long ready, so the PE
                # passes through them without stalling)
                for t in range(4 * (nb - 2), 4 * (nb - 1)):
                    _vtok_one(nc, tp1, g, t)
            ncols = slice(nb * 512, (nb + 1) * 512)
            st_ps = p1p.tile([1, 512], F32, name="st_ps", tag="st_ps")
            # one psum tile per output block so dependency tracking stays
            # per-bank: the next token block's matmul for slot m waits only
            # on slot m's evacuation copy
            mm_ps = [p1p.tile([128, 512], F32, name=f"mm_ps{m}", tag=f"mm_ps{m}")
                     for m in range(QH + 2)]
            wqk = None
            for k in range(KH):
                # weights stream alongside the activations two k-tiles per
                # DMA (re-read per block; DMA has slack and this keeps SBUF
                # residency low)
                if k % 2 == 0:
                    wqk = p1s.tile([128, 2, (QH + 2) * 128], F16, name="wqk",
                                   tag="wqk", bufs=3)
                    nc.sync.dma_start(wqk[:], g["wqkv"][:, k:k + 2, :])
                hb = p1s.tile([128, 512], F16, name="hb", tag="hb", bufs=5)
                nc.sync.dma_start(hb[:], g["hT"][k * 128:(k + 1) * 128, ncols])
                sq = p1s.tile([128, 512], F16, name="sq", tag="sq", bufs=3)
                nc.scalar.activation(sq[:], hb[:], AF.Square)
                nc.tensor.matmul(st_ps[:], g["ones"][:], sq[:],
                                 start=(k == 0), stop=(k == KH - 1))
                for m in range(QH + 2):
                    nc.tensor.matmul(
                        mm_ps[m][:], wqk[:, k % 2, m * 128:(m + 1) * 128], hb[:],
                        start=(k == 0), stop=(k == KH - 1),
                    )
                if k == 20:
                    # rope tables for this block, needed right after the
                    # k-loop (spread across blocks to keep DMA bursts small)
                    nc.sync.dma_start(g["cos_sb"][:, ncols], g["cosT"][:, ncols])
                    nc.sync.dma_start(g["sin_sb"][:, ncols], g["sinT"][:, ncols])
                    if nb == 0:
                        nc.sync.dma_start(g["mask_sb"][:], g["masks"][:, :])
            # evacuate all 6 psum banks immediately (scalar engine) so the
            # next block's matmuls only wait on these copies, not on the
            # rmsnorm/rope chain below
            qkc = p1s.tile([128, QH + 2, 512], F16, name="qkc", tag="qkc", bufs=1)
            for m in range(QH + 2):
                nc.vector.tensor_copy(qkc[:, m, :], mm_ps[m][:])
            # rstd = exp(-0.5*ln(var+eps)): scalar-engine only, and the
            # ln+exp+square table covers every activation until phase 4
            lnv = p1s.tile([1, 512], F32, name="lnv", tag="lnv")
            nc.scalar.activation(lnv[:], st_ps[:], AF.Ln,
                                 bias=g["epsb"][:], scale=1.0 / H)
            rstd16 = p1s.tile([1, 512], F16, name="rstd16", tag="rstd16")
            nc.scalar.activation(rstd16[:], lnv[:], AF.Exp, scale=-0.5)
            rb = p1s.tile([128, 512], F16, name="rb", tag="rb", bufs=1)
            nc.gpsimd.partition_broadcast(rb[:], rstd16[:])
            cos_s = p1s.tile([128, 512], F16, name="cos_s", tag="cos_s", bufs=1)
            nc.vector.tensor_mul(cos_s[:], g["cos_sb"][:, ncols], rb[:])
            sin_s = p1s.tile([128, 512], F16, name="sin_s", tag="sin_s", bufs=1)
            nc.vector.tensor_mul(sin_s[:], g["sin_sb"][:, ncols], rb[:])
            for m in range(QH + 1):
                if m < QH:
                    d0 = g[f"qT{nb}"][0:64, m, :]
                    d1 = g[f"qT{nb}"][64:128, m, :]
                else:
                    d0 = g[f"kT{nb}"][0:64, :]
                    d1 = g[f"kT{nb}"][64:128, :]
                t0 = p1s.tile([64, 512], F16, name="t0", tag="t0", bufs=1)
                nc.vector.tensor_mul(t0[:], qkc[0:64, m, :], cos_s[0:64, :])
                t1 = p1s.tile([64, 512], F16, name="t1", tag="t1", bufs=1)
                nc.vector.tensor_mul(t1[:], qkc[64:128, m, :], sin_s[64:128, :])
                nc.vector.tensor_sub(d0, t0[:], t1[:])
                t2 = p1s.tile([64, 512], F16, name="t2", tag="t0", bufs=1)
                nc.vector.tensor_mul(t2[:], qkc[64:128, m, :], cos_s[64:128, :])
                t3 = p1s.tile([64, 512], F16, name="t3", tag="t1", bufs=1)
                nc.vector.tensor_mul(t3[:], qkc[0:64, m, :], sin_s[0:64, :])
                nc.vector.tensor_add(d1, t2[:], t3[:])
            nc.vector.tensor_mul(g[f"vT{nb}"][:, :], qkc[:, QH + 1, :], rb[:])

        # V transposes for blocks 2-3 happen in phase 2 so their rmsnorm
        # chains never stall the PE at the phase boundary


def _vtok_one(nc, tp, g, t):
    """Transpose one 128-token V tile to token-major for the PV matmuls."""
    nb, j = t // 4, t % 4
    nc.tensor.transpose(tp[:, t % 8, :], g[f"vT{nb}"][:, j * 128:(j + 1) * 128],
                        g["ident"][:])
    nc.vector.tensor_copy(g["v_tok"][:, t, :], tp[:, t % 8, :])


def _phase2_attention(nc, tc, g, with_collectives, rg):
    with (
        tc.tile_pool(name="p2sbuf", bufs=2) as p2s,
        tc.tile_pool(name="p2psum", bufs=1, space="PSUM") as p2p,
    ):
        # prefetch phase-3/4/5 operands with no dependency on attention
        # (phase 2 has plenty of DMA slack)
        nc.sync.dma_start(g["hsl"][:], g["hT_slice"][:, :, :])
        for m in range(3):
            nc.sync.dma_start(g["wo_pre"][:, m, :, :], g["wo"][:, m, :, :])
        nc.sync.dma_start(g["gu_pre"][:], g["wgu"][:, 0, :, :])
        nc.sync.dma_start(g["wdn_pre"][:], g["wdn"][:, 0, :, :])

        for hh in range(QH):
            anob = p2s.tile([128, NB, 512], F16, name="anob", tag="anob", bufs=2)
            for qb in range(NB):
                if hh == 0 and qb in (1, 2):
                    # blocks 2/3's V transposes: emitted one block before
                    # their first use so the DVE copies complete in time
                    tp2 = p2p.tile([128, 8, 128], F16, name="tp2", tag="tp2")
                    for t in range(4 + 4 * qb, 8 + 4 * qb):
                        _vtok_one(nc, tp2, g, t)
                qcols = slice(qb * 512, (qb + 1) * 512)
                att_ps = p2p.tile([128, 512], F32, name="att_ps", tag="att_ps", bufs=2)
                acc = p2s.tile([128, 2, 512], F16, name="acc", tag="acc", bufs=2)
                # full (unmasked) score tiles, processed in pairs
                for pp in range(2 * qb):
                    kt0, kt1 = 2 * pp, 2 * pp + 1
                    s_ps = p2p.tile([128, 2, 512], F32, name="s_ps", tag="s_ps", bufs=2)
                    nc.tensor.matmul(
                        s_ps[:, 0, :],
                        g[f"kT{kt0 // 4}"][:, (kt0 % 4) * 128:(kt0 % 4 + 1) * 128],
                        g[f"qT{qb}"][:, hh, :], start=True, stop=True,
                    )
                    nc.tensor.matmul(
                        s_ps[:, 1, :],
                        g[f"kT{kt1 // 4}"][:, (kt1 % 4) * 128:(kt1 % 4 + 1) * 128],
                        g[f"qT{qb}"][:, hh, :], start=True, stop=True,
                    )
                    e = p2s.tile([128, 2, 512], F16, name="e", tag="e", bufs=6)
                    nc.scalar.activation(e[:], s_ps[:], AF.Exp, scale=SCALE)
                    if pp == 0:
                        nc.vector.tensor_copy(acc[:], e[:])
                    else:
                        nc.vector.tensor_add(acc[:], acc[:], e[:])
                    nc.tensor.matmul(att_ps[:], g["v_tok"][:, kt0, :], e[:, 0, :],
                                     start=(pp == 0), stop=False)
                    nc.tensor.matmul(att_ps[:], g["v_tok"][:, kt1, :], e[:, 1, :],
                                     start=False, stop=False)
                # the 4 diagonal tiles: causality restricts tile j to local
                # q >= 128j, so matmul/exp/accumulate only the valid width and
                # apply a 128x128 triangle mask to the leading sub-block
                for pj in range(2):
                    s_ps = p2p.tile([128, 2, 512], F32, name="s_ps", tag="s_ps", bufs=2)
                    if hh == 0 and qb == 0:
                        # first score block: tiles live in the attention pool
                        # (fresh region), so the first exps carry no WAR
                        # against phase-1's rope tiles
                        e = g["e0a"] if pj == 0 else g["e0b"]
                    else:
                        e = p2s.tile([128, 2, 512], F16, name="e", tag="e", bufs=6)
                    for i in range(2):
                        j = 2 * pj + i
                        qoff, w = 128 * j, 512 - 128 * j
                        nc.tensor.matmul(
                            s_ps[:, i, 0:w],
                            g[f"kT{qb}"][:, j * 128:(j + 1) * 128],
                            g[f"qT{qb}"][:, hh, qoff:512],
                            start=True, stop=True,
                        )
                    if pj == 0:
                        # j=0 is full width; j=1 region [384:512) holds stale
                        # psum, exp'd but never read
                        nc.scalar.activation(e[:], s_ps[:], AF.Exp, scale=SCALE)
                    else:
                        nc.scalar.activation(e[:, 0, 0:256], s_ps[:, 0, 0:256],
                                             AF.Exp, scale=SCALE)
                        nc.scalar.activation(e[:, 1, 0:128], s_ps[:, 1, 0:128],
                                             AF.Exp, scale=SCALE)
                    for i in range(2):
                        j = 2 * pj + i
                        kt = 4 * qb + j
                        qoff, w = 128 * j, 512 - 128 * j
                        nc.vector.tensor_mul(e[:, i, 0:128], e[:, i, 0:128],
                                             g["mask_sb"][:, :])
                        if qb == 0 and j == 0:
                            nc.vector.tensor_copy(acc[:, 0, :], e[:, 0, :])
                        elif qb == 0 and j == 1:
                            nc.gpsimd.memset(acc[:, 1, 0:128], 0.0)
                            nc.vector.tensor_copy(acc[:, 1, 128:512], e[:, 1, 0:384])
                        else:
                            nc.vector.tensor_add(acc[:, i, qoff:512],
                                                 acc[:, i, qoff:512], e[:, i, 0:w])
                        nc.tensor.matmul(
                            att_ps[:, qoff:512], g["v_tok"][:, kt, :], e[:, i, 0:w],
                            start=(qb == 0 and j == 0), stop=(j == 3),
                        )
                sums_ps = p2p.tile([1, 512], F32, name="sums_ps", tag="sums_ps", bufs=1)
                nc.tensor.matmul(sums_ps[:], g["ones"][:], acc[:, 0, :],
                                 start=True, stop=False)
                nc.tensor.matmul(sums_ps[:], g["ones"][:], acc[:, 1, :],
                                 start=False, stop=True)
                recip = p2s.tile([1, 512], F32, name="recip", tag="recip")
                nc.vector.reciprocal(recip[:], sums_ps[:])
                recip16 = p2s.tile([1, 512], F16, name="recip16", tag="recip16")
                nc.vector.tensor_copy(recip16[:], recip[:])
                rb2 = p2s.tile([128, 512], F16, name="rb2", tag="rb2", bufs=2)
                nc.gpsimd.partition_broadcast(rb2[:], recip16[:])
                nc.vector.tensor_mul(anob[:, qb, :], att_ps[:], rb2[:])
            # one batched DMA scatters this head's context into the A2A
            # input layout ([core, 128, 256]); per-partition runs stay 512B
            nc.sync.dma_start(
                g[f"a2a_in{hh}"][:, :, :].transpose([1, 0, 2]),
                anob[:, :, :],
            )
            # ship this head's context while the next head computes
            if with_collectives:
                nc.gpsimd.collective_compute(
                    "AllToAll", mybir.AluOpType.bypass, replica_groups=rg,
                    ins=[g[f"a2a_in{hh}"].opt()], outs=[g[f"a2a_out{hh}"].opt()],
                )
                # pull this head's context into the o_proj operand layout
                nc.sync.dma_start(
                    g["asl"][:, hh, :, :],
                    g[f"a2a_out{hh}"][:, :, :].transpose([1, 0, 2]),
                )
            else:
                # local stand-in: same byte count, sourced from SBUF so the
                # o_proj operand isn't chained behind the a2a_in write
                nc.sync.dma_start(g["asl"][:, hh, :, :], anob[:, :, :])


def _phase3_oproj(nc, tc, g, with_collectives, rg):
    with (
        tc.tile_pool(name="p3sbuf", bufs=2) as p3s,
        tc.tile_pool(name="p3psum", bufs=1, space="PSUM") as p3p,
    ):
        res2 = p3s.tile([128, KH, TPC], F16, name="res2", tag="res2", bufs=1)
        st2_ps = p3p.tile([1, TPC], F32, name="st2_ps", tag="st2_ps")
        asl = g["asl"]
        for m in range(KH):
            if m < 3:
                wsrc = g["wo_pre"][:, m]
            else:
                wob = p3s.tile([128, KH, 128], F16, name="wob", tag="wob", bufs=3)
                nc.sync.dma_start(wob[:], g["wo"][:, m, :, :])
                wsrc = wob
            o_ps = p3p.tile([128, TPC], F32, name="o_ps", tag="o_ps", bufs=2)
            for k in range(KH):
                nc.tensor.matmul(o_ps[:], wsrc[:, k, :], asl[:, k // 8, k % 8, :],
                                 start=(k == 0), stop=(k == KH - 1))
            nc.vector.tensor_add(res2[:, m, :], o_ps[:], g["hsl"][:, m, :])
            nc.sync.dma_start(g["res_out"][m * 128:(m + 1) * 128, :], res2[:, m, :])
            sq2 = p3s.tile([128, TPC], F16, name="sq2", tag="sq2", bufs=2)
            nc.vector.tensor_mul(sq2[:], res2[:, m, :], res2[:, m, :])
            nc.tensor.matmul(st2_ps[:], g["ones"][:], sq2[:],
                             start=(m == 0), stop=(m == KH - 1))
        lnv2 = p3s.tile([1, TPC], F32, name="lnv2", tag="lnv2")
        nc.scalar.activation(lnv2[:], st2_ps[:], AF.Ln, bias=g["epsb"][:], scale=1.0 / H)
        rstd2_16 = p3s.tile([1, TPC], F16, name="rstd2_16", tag="rstd2_16")
        nc.scalar.activation(rstd2_16[:], lnv2[:], AF.Exp, scale=-0.5)
        rb3 = p3s.tile([128, TPC], F16, name="rb3", tag="rb3")
        nc.gpsimd.partition_broadcast(rb3[:], rstd2_16[:])
        x2bs = []
        for kq in range(4):
            # batch 8 feature-tiles of normed activations into one DMA, then
            # ship the quarter so phase 4's operands stream in incrementally
            x2b = p3s.tile([128, 8, TPC], F16, name="x2b", tag="x2b", bufs=4)
            x2bs.append(x2b)
            for mi in range(8):
                nc.vector.tensor_mul(x2b[:, mi, :], res2[:, kq * 8 + mi, :], rb3[:])
            if with_collectives:
                nc.sync.dma_start(g[f"ag2_in_q{kq}"][:, :, :], x2b[:, :, :])
                nc.gpsimd.collective_compute(
                    "AllGather", mybir.AluOpType.bypass, replica_groups=rg,
                    ins=[g[f"ag2_in_q{kq}"].opt()],
                    outs=[g[f"ag2_out_q{kq}"].opt()],
                )
        g["x2bs"] = x2bs


def _phase4_gate_up(nc, tc, g, with_collectives):
    """SwiGLU first half, one 512-token quarter at a time.

    Each quarter's activations (4.2MB fp16) double-buffer against the previous
    quarter's compute; gate/up weights re-stream per quarter (DMA has slack).
    h = silu(gate)*up is written straight to the SBUF-resident hful tile, so
    phase 5 starts with everything already on-chip.
    """
    with (
        tc.tile_pool(name="p4x", bufs=1) as p4x,
        tc.tile_pool(name="p4sbuf", bufs=2) as p4s,
        tc.tile_pool(name="p4psum", bufs=1, space="PSUM") as p4p,
    ):
        for tb in range(NB):
            tcols = slice(tb * 512, (tb + 1) * 512)
            if tb % 2 == 0:
                # x2q0 sits in the region freed by the attention pools, so
                # its first-quarter write has no WAR against live phase-3
                # tiles; odd quarters ping-pong through the phase-4 pool
                x2q = g["x2q0"]
            else:
                x2q = p4x.tile([128, KH, 512], F16, name="x2q", tag="x2q", bufs=1)
            if with_collectives or tb > 0:
                for kq in range(4):
                    # one DMA per feature-quarter (both source cores adjacent
                    # in the gather buffer) so the k-loop starts on quarter 0
                    nc.sync.dma_start(
                        x2q[:, kq * 8:(kq + 1) * 8, :],
                        g[f"ag2_out_q{kq}"][2 * tb:2 * tb + 2, :, :, :]
                        .transpose([1, 2, 0, 3]),
                    )
            else:
                # local stand-in for the gather with the same per-core DMA
                # byte count as the real path: this core's slice straight
                # from SBUF, the peer slice from the (unwritten) gather buffer
                for kq in range(4):
                    nc.sync.dma_start(x2q[:, kq * 8:(kq + 1) * 8, 0:256],
                                      g["x2bs"][kq][:, :, :])
                    nc.sync.dma_start(x2q[:, kq * 8:(kq + 1) * 8, 256:512],
                                      g[f"ag2_out_q{kq}"][1, :, :, :])
            if tb == 1 and not with_collectives:
                # collective-input writes (same DMA bytes as the real path's
                # kernel side), deferred behind the first quarters' operands
                for kq in range(4):
                    nc.sync.dma_start(g[f"ag2_in_q{kq}"][:, :, :],
                                      g["x2bs"][kq][:, :, :])
                    nc.sync.dma_start(g[f"ag2_out_q{kq}"][0, :, :, :],
                                      g["x2bs"][kq][:, :, :])
            for m in range(MB_GU):
                if m == 0:
                    gu = g["gu_pre"]
                else:
                    gu = p4s.tile([128, KH, 256], F16, name="gu", tag="gu", bufs=2)
                    nc.sync.dma_start(gu[:], g["wgu"][:, m, :, :])
                g_ps = p4p.tile([128, 512], F32, name="g_ps", tag="g_ps", bufs=2)
                for k in range(KH):
                    nc.tensor.matmul(g_ps[:], gu[:, k, 0:128], x2q[:, k, :],
                                     start=(k == 0), stop=(k == KH - 1))
                u_ps = p4p.tile([128, 512], F32, name="u_ps", tag="u_ps", bufs=2)
                for k in range(KH):
                    nc.tensor.matmul(u_ps[:], gu[:, k, 128:256], x2q[:, k, :],
                                     start=(k == 0), stop=(k == KH - 1))
                sg = p4s.tile([128, 512], F16, name="sg", tag="sg", bufs=2)
                nc.scalar.activation(sg[:], g_ps[:], AF.Silu)
                nc.vector.tensor_mul(g["hful"][:, m, tcols], sg[:], u_ps[:])


def _phase5_down(nc, tc, g, with_collectives, rg):
    with (
        tc.tile_pool(name="p5sbuf", bufs=2) as p5s,
        tc.tile_pool(name="p5psum", bufs=1, space="PSUM") as p5p,
    ):
        hful = g["hful"]
        for r in range(8):
            for mi in range(KH // 8):
                m = r * (KH // 8) + mi
                if m == 0:
                    db = g["wdn_pre"]
                else:
                    db = p5s.tile([128, KI, 128], F16, name="db", tag="db", bufs=3)
                    nc.sync.dma_start(db[:], g["wdn"][:, m, :, :])
                for tb in range(NB):
                    tcols = slice(tb * 512, (tb + 1) * 512)
                    d_ps = p5p.tile([128, 512], F32, name="d_ps", tag="d_ps", bufs=2)
                    for k in range(KI):
                        nc.tensor.matmul(d_ps[:], db[:, k, :], hful[:, k, tcols],
                                         start=(k == 0), stop=(k == KI - 1))
                    ot = p5s.tile([128, 512], F16, name="ot", tag="ot", bufs=2)
                    nc.vector.tensor_copy(ot[:], d_ps[:])
                    nc.sync.dma_start(g[f"rs_in{r}"][mi * 128:(mi + 1) * 128, tcols], ot[:])
            if with_collectives:
                nc.gpsimd.collective_compute(
                    "ReduceScatter", mybir.AluOpType.add, replica_groups=rg,
                    ins=[g[f"rs_in{r}"].opt()], outs=[g[f"rs_out{r}"].opt()],
                )
            else:
                nc.sync.dma_start(g[f"rs_out{r}"][:, :], g[f"rs_in{r}"][0:H // NC // 8, :])
            nc.sync.dma_start(
                g["out_down"][r * 64:(r + 1) * 64, :], g[f"rs_out{r}"][:, :])


def build_program(with_collectives=True, stop_after=99):
    nc = bacc.Bacc("TRN2", target_bir_lowering=False, debug=False, num_devices=NC)

    g = {}
    g["hT"] = nc.dram_tensor("hT", [H, S], F16, kind="ExternalInput")
    g["hT_slice"] = nc.dram_tensor("hT_slice", [128, KH, TPC], F16, kind="ExternalInput")
    g["wqkv"] = nc.dram_tensor("wqkv", [128, KH, (QH + 2) * 128], F16, kind="ExternalInput")
    g["wo"] = nc.dram_tensor("wo", [128, KH, KH, 128], F16, kind="ExternalInput")
    g["wgu"] = nc.dram_tensor("wgu", [128, MB_GU, KH, 256], F16, kind="ExternalInput")
    g["wdn"] = nc.dram_tensor("wdn", [128, KH, KI, 128], F16, kind="ExternalInput")
    g["cosT"] = nc.dram_tensor("cosT", [128, S], F16, kind="ExternalInput")
    g["sinT"] = nc.dram_tensor("sinT", [128, S], F16, kind="ExternalInput")
    g["masks"] = nc.dram_tensor("masks", [128, 128], F16, kind="ExternalInput")

    g["res_out"] = nc.dram_tensor("res_out", [H, TPC], F16, kind="ExternalOutput")
    g["out_down"] = nc.dram_tensor("out_down", [H // NC, S], F16, kind="ExternalOutput")

    rg = [list(range(NC))]

    with tile.TileContext(nc) as tc:
        with (
            tc.tile_pool(name="consts", bufs=1) as consts,
            tc.tile_pool(name="dram", bufs=1, space="DRAM") as dram,
        ):
            for hh in range(QH):
                g[f"a2a_in{hh}"] = dram.tile([NC, 128, TPC], F16, name=f"a2a_in{hh}")
                g[f"a2a_out{hh}"] = dram.tile([NC, 128, TPC], F16, name=f"a2a_out{hh}")
            for kq in range(4):
                g[f"ag2_in_q{kq}"] = dram.tile([128, 8, TPC], F16, name=f"ag2_in_q{kq}")
                g[f"ag2_out_q{kq}"] = dram.tile([NC, 128, 8, TPC], F16,
                                                name=f"ag2_out_q{kq}", addr_space="Shared")
            for r in range(8):
                g[f"rs_in{r}"] = dram.tile([H // 8, S], F16, name=f"rs_in{r}")
                g[f"rs_out{r}"] = dram.tile([H // NC // 8, S], F16, name=f"rs_out{r}")

            ones32 = consts.tile([128, 1], F32, name="ones32")
            nc.gpsimd.memset(ones32[:], 1.0)
            g["ones"] = consts.tile([128, 1], F16, name="ones")
            nc.vector.tensor_copy(g["ones"][:], ones32[:])
            ident32 = consts.tile([128, 128], F32, name="ident32")
            make_identity(nc, ident32[:])
            g["ident"] = consts.tile([128, 128], F16, name="ident")
            nc.vector.tensor_copy(g["ident"][:], ident32[:])
            g["epsb"] = consts.tile([1, 1], F32, name="epsb")
            nc.gpsimd.memset(g["epsb"][:], EPS)
            g["dummy"] = consts.tile([1, 1], F32, name="dummy")
            nc.scalar.add_instruction(mybir.InstLoadActFuncSet(
                name=nc.get_next_instruction_name(), act_func_set_id=6,
                ins=[], outs=[]))

            # h = silu(gate)*up stays SBUF-resident across phases 4 and 5
            with tc.tile_pool(name="mlpkeep", bufs=1) as mlpkeep:
                g["hful"] = mlpkeep.tile([128, KI, S], F16, name="hful")  # 7.3 MB
                g["wdn_pre"] = mlpkeep.tile([128, KI, 128], F16, name="wdn_pre")
                g["gu_pre"] = mlpkeep.tile([128, KH, 256], F16, name="gu_pre")  # 2 MB

                # phase-3 operands that outlive the attention pools
                with tc.tile_pool(name="p3keep", bufs=1) as p3keep:
                    g["hsl"] = p3keep.tile([128, KH, TPC], F16, name="hsl")       # 2 MB
                    g["asl"] = p3keep.tile([128, QH, NC, TPC], F16, name="asl")   # 2 MB
                    g["wo_pre"] = p3keep.tile([128, 3, KH, 128], F16, name="wo_pre")  # 3 MB

                    with tc.tile_pool(name="attn", bufs=1) as attn:
                        g["cos_sb"] = attn.tile([128, S], F16, name="cos_sb")
                        g["sin_sb"] = attn.tile([128, S], F16, name="sin_sb")
                        g["mask_sb"] = attn.tile([128, 128], F16, name="mask_sb")
                        for nb in range(NB):
                            g[f"qT{nb}"] = attn.tile([128, QH, 512], F16, name=f"qT{nb}")
                            g[f"kT{nb}"] = attn.tile([128, 512], F16, name=f"kT{nb}")
                        for nb in range(NB):
                            g[f"vT{nb}"] = attn.tile([128, 512], F16, name=f"vT{nb}")
                        g["v_tok"] = attn.tile([128, S // 128, 128], F16, name="v_tok")  # 0.5 MB
                        g["e0a"] = attn.tile([128, 2, 512], F16, name="e0a")
                        g["e0b"] = attn.tile([128, 2, 512], F16, name="e0b")

                        _phase1_qkv(nc, tc, g)
                        if stop_after >= 2:
                            _phase2_attention(nc, tc, g, with_collectives, rg)

                    if stop_after >= 3:
                        _phase3_oproj(nc, tc, g, with_collectives, rg)

                with tc.tile_pool(name="xq0", bufs=1) as xq0:
                    g["x2q0"] = xq0.tile([128, KH, 512], F16, name="x2q0")  # 4.2 MB

                    if stop_after >= 4:
                        _phase4_gate_up(nc, tc, g, with_collectives)

                if stop_after >= 5:
                    _phase5_down(nc, tc, g, with_collectives, rg)

    nc.finalize()
    return nc


_cached_nc = None


def _get_nc():
    global _cached_nc
    if _cached_nc is None:
        _cached_nc = build_program(with_collectives=True)
    return _cached_nc


def _host_prep(positions, hidden_states, w_qkv, w_o, w_gate_up, w_down, ln1_w, ln2_w):
    f32 = np.float32
    f16 = np.float16
    hidden = np.asarray(hidden_states, dtype=f32)[0]          # [S, H]
    hT = np.ascontiguousarray(hidden.T).astype(f16)            # [H, S]
    pos = np.asarray(positions).astype(f32)[0]                 # [S]

    half = HD // 2
    inv_freq = (1.0 / (f32(THETA) ** (np.arange(0, half, dtype=f32) / f32(half)))).astype(f32)
    ang = pos[:, None] * inv_freq[None, :]                     # [S, 64] fp32
    cos_half = np.cos(ang).astype(f32).T                       # [64, S]
    sin_half = np.sin(ang).astype(f32).T
    cosT_np = np.concatenate([cos_half, cos_half], axis=0).astype(f16)  # [128, S]
    sinT_np = np.concatenate([sin_half, sin_half], axis=0).astype(f16)

    w_qkv_f = np.asarray(w_qkv, dtype=f32) * np.asarray(ln1_w, dtype=f32)[:, None]
    w_gu_f = np.asarray(w_gate_up, dtype=f32) * np.asarray(ln2_w, dtype=f32)[:, None]
    # contraction (k') order is head-major: k' = hh*8 + r <-> global head 4r+hh
    kperm = [4 * (k % NC) + (k // NC) for k in range(KH)]
    w_o_f = np.ascontiguousarray(
        np.asarray(w_o, dtype=f32).reshape(KH, 128, KH, 128)
        .transpose(1, 2, 0, 3)[:, :, kperm, :]
    ).astype(f16)
    w_dn_f = np.asarray(w_down, dtype=f32)

    # causal triangle for the leading 128 columns of each diagonal tile
    masks_np = np.ascontiguousarray(
        (np.arange(128)[None, :] >= np.arange(128)[:, None]).astype(f16))  # [128, 128]

    in_maps = []
    for c in range(NC):
        q_cols = w_qkv_f[:, c * QH * HD:(c + 1) * QH * HD]
        k_col = w_qkv_f[:, NQ * HD + c * HD: NQ * HD + (c + 1) * HD]
        v_col = w_qkv_f[:, (NQ + NKV) * HD + c * HD: (NQ + NKV) * HD + (c + 1) * HD]
        wqkv_c = np.concatenate([q_cols, k_col, v_col], axis=1)
        wqkv_c = np.ascontiguousarray(
            wqkv_c.reshape(KH, 128, (QH + 2) * 128).transpose(1, 0, 2)).astype(f16)
        # per-m interleave: [128, m, k, gate128|up128]
        wg_c = w_gu_f[:, c * IPC:(c + 1) * IPC].reshape(KH, 128, MB_GU, 128)
        wu_c = w_gu_f[:, I + c * IPC: I + (c + 1) * IPC].reshape(KH, 128, MB_GU, 128)
        wgu_c = np.ascontiguousarray(
            np.concatenate([wg_c[..., None, :], wu_c[..., None, :]], axis=3)
            .reshape(KH, 128, MB_GU, 256).transpose(1, 2, 0, 3)).astype(f16)
        wdn_c = np.ascontiguousarray(
            w_dn_f[c * IPC:(c + 1) * IPC, :].reshape(KI, 128, KH, 128)
            .transpose(1, 2, 0, 3)).astype(f16)
        hT_slice_c = np.ascontiguousarray(
            hT[:, c * TPC:(c + 1) * TPC].reshape(KH, 128, TPC).transpose(1, 0, 2))
        in_maps.append({
            "hT": hT,
            "hT_slice": hT_slice_c,
            "wqkv": wqkv_c,
            "wo": w_o_f,
            "wgu": wgu_c,
            "wdn": wdn_c,
            "cosT": cosT_np,
            "sinT": sinT_np,
            "masks": masks_np,
        })
    return in_maps


def kernel(**inputs):
    in_maps = _host_prep(**inputs)
    nc = _get_nc()
    res = run_bass_kernel_spmd(nc, in_maps, core_ids=list(range(NC)))
    results = res.results

    outT = np.empty((H, S), np.float32)
    for c in range(NC):
        od = results[c]["out_down"].astype(np.float32)  # [512, S]
        for r in range(8):
            outT[512 * r + 64 * c: 512 * r + 64 * (c + 1)] = od[64 * r:64 * (r + 1)]
    resT = np.concatenate(
        [results[c]["res_out"].astype(np.float32) for c in range(NC)], axis=1)  # [H, S]
    out = np.ascontiguousarray(outT.T).reshape(1, S, H).astype(np.float32)
    residual = np.ascontiguousarray(resT.T).reshape(1, S, H).astype(np.float32)
    return out, residual
